# revision 1
# baseline (speedup 1.0000x reference)
"""CBAM kernel for Trainium2, 8-way batch-parallel SPMD.

Computes out = x^2 * (att_c[b,c] + sigmoid(conv(spatial_stats))[b,l]) where
att_c = sigmoid(mlp(mean_L x) + mlp(max_L x)), matching the CBAM reference.

Layout per core: 4 batches; each batch x[4096, 256] lives in SBUF as one
[128, 8192] tensor (partition = l % 128, free column = 256*(l//128) + c).
Engine split per batch:
  PE   : channel-sum (ones-matmul accumulation), transposes, MLP, conv
         (conv over L is a banded-Toeplitz matmul with host-built weights)
  ACT  : spatial sums (copy w/ accum_out), squares, sigmoids, relu
  DVE  : spatial max (one 3D reduce), max-tree folds, final fused
         (att + sig) * x^2 via scalar_tensor_tensor
  POOL : first max-tree fold
"""

import numpy as np
from contextlib import ExitStack

import concourse.bacc as bacc
import concourse.bass as bass
import concourse.tile as tile
import concourse.mybir as mybir
from concourse.bass_utils import run_bass_kernel_spmd

AF = mybir.ActivationFunctionType
ALU = mybir.AluOpType
AX = mybir.AxisListType
FP32 = mybir.dt.float32

N_CORES = 8
B_FULL = 32
NB = B_FULL // N_CORES  # batches per core = 4
L = 4096
C = 256
HID = 16
P = 128
NT = L // P  # 32 L-tiles per batch
SQW = 2048   # ACT square slice width (8 tiles)

_CACHE: dict = {}


def _build_body(ctx: ExitStack, tc, out_d, x_d, w1_d, b1_d, w2b_d, cm_d, cc_d,
                ones_d, id_d, rc_d, reps=1):
    nc = tc.nc

    const = ctx.enter_context(tc.tile_pool(name="const", bufs=1))
    xpool = ctx.enter_context(tc.tile_pool(name="x", bufs=2))
    mpool = ctx.enter_context(tc.tile_pool(name="maxtree", bufs=2))
    spool = ctx.enter_context(tc.tile_pool(name="stats", bufs=2))
    sqpool = ctx.enter_context(tc.tile_pool(name="sq", bufs=3))
    opool = ctx.enter_context(tc.tile_pool(name="outt", bufs=8))
    dpool = ctx.enter_context(tc.tile_pool(name="dummy", bufs=2))
    apool = ctx.enter_context(tc.tile_pool(name="att", bufs=2))
    pacc = ctx.enter_context(tc.tile_pool(name="pacc", bufs=2, space="PSUM"))
    pwork = ctx.enter_context(tc.tile_pool(name="pwork", bufs=4, space="PSUM"))

    w1 = const.tile([P, 2 * (HID + 1)], FP32)
    nc.sync.dma_start(w1[:], w1_d[:])
    b1 = const.tile([HID + 1, 1], FP32)
    nc.sync.dma_start(b1[:], b1_d[:])
    w2b = const.tile([HID + 1, C], FP32)
    nc.sync.dma_start(w2b[:], w2b_d[:])
    cmain = const.tile([P, 2 * P], FP32)
    nc.sync.dma_start(cmain[:], cm_d[:])
    ccorn = const.tile([P, 4 * P], FP32)
    nc.sync.dma_start(ccorn[:], cc_d[:])
    ones = const.tile([P, P], FP32)
    nc.sync.dma_start(ones[:], ones_d[:])
    ident = const.tile([P, P], FP32)
    nc.sync.dma_start(ident[:], id_d[:])
    redcol = const.tile([P, 1], FP32)
    nc.sync.dma_start(redcol[:], rc_d[:])

    HN = NT // 2  # 16 tiles per half-batch
    for b in [b for _ in range(reps) for b in range(NB)]:
        # x lives in two half-batch tensors so compute starts once the
        # first half has landed.
        xh = [xpool.tile([P, HN * C], FP32, tag=f"xb{h}", name=f"xb{h}")
              for h in range(2)]
        for t in range(NT):
            nc.sync.dma_start(xh[t // HN][:, C * (t % HN):C * (t % HN + 1)],
                              x_d[b, P * t:P * (t + 1), :])

        def xtile(t):
            return xh[t // HN][:, C * (t % HN):C * (t % HN + 1)]

        # ---- channel mean over L (PE): lhsT = x tile half, rhs = 1/L col;
        # psum [128, 1] per channel-half accumulates in channel-major ----
        # lhsT = 1/L column (stationary, loaded once), x streams as rhs:
        # pcs[0, c] accumulates mean over L.
        pcs = pacc.tile([1, C], FP32, tag="pcs")
        for t in range(NT):
            nc.tensor.matmul(pcs[:], redcol[:], xtile(t)[:],
                             start=(t == 0), stop=(t == NT - 1),
                             skip_group_check=True)

        # ---- spatial sum over C: ACT copies with accum_out ----
        sum_s = spool.tile([P, NT], FP32, tag="sum_s")
        for t in range(NT):
            dummy = dpool.tile([P, C], FP32, tag="dummy")
            nc.scalar.activation(dummy[:], xtile(t)[:],
                                 AF.Identity, accum_out=sum_s[:, t:t + 1])

        # ---- spatial max over C: one 3D reduce (DVE) per half ----
        max_s = spool.tile([P, NT], FP32, tag="max_s")
        for h in range(2):
            nc.vector.tensor_reduce(
                max_s[:, HN * h:HN * (h + 1)],
                xh[h][:].rearrange("p (t c) -> p t c", c=C),
                axis=AX.X, op=ALU.max)

        # ---- channel max over L: fold tree then transpose+reduce ----
        half = HN * C  # 4096
        mb = mpool.tile([P, half], FP32, tag="mb")
        nc.vector.tensor_max(mb[:], xh[0][:], xh[1][:])
        w = half // 2
        while w >= C:
            nc.vector.tensor_max(mb[:, 0:w], mb[:, 0:w], mb[:, w:2 * w])
            w //= 2

        stats_cm = spool.tile([P, 4], FP32, tag="stats_cm")
        avg_row = spool.tile([1, C], FP32, tag="avg_row")
        nc.scalar.activation(avg_row[:], pcs[:], AF.Copy)
        # chan-max without PE transposes: 32x32 block transpose (DVE) puts
        # channels along free within blocks; reduce in-block, then fold the
        # four partition quadrants.
        bt = spool.tile([P, C], FP32, tag="bt")
        nc.vector.transpose(bt[:], mb[:, 0:C])
        red = spool.tile([P, 8], FP32, tag="red")
        nc.vector.tensor_reduce(red[:],
                                bt[:].rearrange("p (bj s) -> p bj s", s=32),
                                axis=AX.X, op=ALU.max)
        # DVE ops need equal base partitions, so fold the four partition
        # quadrants by gathering them into columns with tiny DMAs first.
        cm32 = spool.tile([32, 32], FP32, tag="cm32")
        for a in range(4):
            nc.gpsimd.dma_start(cm32[:, 8 * a:8 * (a + 1)],
                                red[32 * a:32 * (a + 1), :])
        cmf = spool.tile([32, 8], FP32, tag="cmf")
        nc.vector.tensor_reduce(cmf[:],
                                cm32[:].rearrange("r (a bj) -> r bj a", a=4),
                                axis=AX.X, op=ALU.max)
        # scatter into channel-major stats via tiny stream-matched DMAs on
        # the idle gpsimd queue: cmf[r, bj] is the max of channel 32*bj+r.
        for h in range(2):
            nc.gpsimd.dma_start(stats_cm[:, 2 * h:2 * h + 1],
                                avg_row[0:1, P * h:P * (h + 1)])
        for bj in range(8):
            q = 32 * (bj % 4)
            nc.gpsimd.dma_start(stats_cm[q:q + 32, 2 * (bj // 4) + 1:
                                         2 * (bj // 4) + 2],
                                cmf[:, bj:bj + 1])

        # ---- shared MLP: att logits broadcast over partitions via matmul ----
        # Row HID (=16) carries a constant: lhsT col 16 is zero, relu bias row
        # 16 is 1.0, so hsb[16, :] = 1, h2[16] = 2 — which multiplies the b2
        # row of w2b to add the 2*b2 term.
        ph = pwork.tile([HID + 1, 2], FP32, tag="pwork")
        nc.tensor.matmul(ph[:], w1[:, 0:HID + 1], stats_cm[:, 0:2],
                         start=True, stop=False, skip_group_check=True)
        nc.tensor.matmul(ph[:], w1[:, HID + 1:2 * (HID + 1)], stats_cm[:, 2:4],
                         start=False, stop=True, skip_group_check=True)
        hsb = spool.tile([HID + 1, 2], FP32, tag="hsb")
        nc.scalar.activation(hsb[:], ph[:], AF.Relu, bias=b1[:])
        h2 = spool.tile([HID + 1, 1], FP32, tag="h2")
        nc.vector.tensor_add(h2[:], hsb[:, 0:1], hsb[:, 1:2])
        h2r = spool.tile([HID + 1, P], FP32, tag="h2r")
        nc.vector.tensor_scalar_mul(h2r[:], ones[0:HID + 1, :], h2[:])
        po = pwork.tile([P, C], FP32, tag="pwork")
        nc.tensor.matmul(po[:], h2r[:], w2b[:], start=True, stop=True,
                         skip_group_check=True)
        att = apool.tile([P, C], FP32, tag="att")
        nc.scalar.activation(att[:], po[:], AF.Sigmoid)

        # ---- spatial conv over L: banded-Toeplitz matmuls ----
        pc = pwork.tile([P, NT], FP32, tag="pwork")
        nc.tensor.matmul(pc[:, :], cmain[:, 0:P], sum_s[:],
                         start=True, stop=False, skip_group_check=True)
        nc.tensor.matmul(pc[:, :], cmain[:, P:2 * P], max_s[:],
                         start=False, stop=False, skip_group_check=True)
        nc.tensor.matmul(pc[:, 1:NT], ccorn[:, 0:P], sum_s[:, 0:NT - 1],
                         start=False, stop=False, skip_group_check=True)
        nc.tensor.matmul(pc[:, 1:NT], ccorn[:, P:2 * P],
                         max_s[:, 0:NT - 1],
                         start=False, stop=False, skip_group_check=True)
        nc.tensor.matmul(pc[:, 0:NT - 1], ccorn[0:3, 2 * P:3 * P],
                         sum_s[0:3, 1:NT],
                         start=False, stop=False, skip_group_check=True)
        nc.tensor.matmul(pc[:, 0:NT - 1], ccorn[0:3, 3 * P:4 * P],
                         max_s[0:3, 1:NT],
                         start=False, stop=True, skip_group_check=True)
        sig = spool.tile([P, NT], FP32, tag="sig")
        nc.scalar.activation(sig[:], pc[:], AF.Sigmoid)

        # ---- final: out = (att + sig) * x^2 ----
        sph = HN * C // SQW  # square slices per half
        sqs = []
        for s in range(NT * C // SQW):
            sq = sqpool.tile([P, SQW], FP32, tag="sq")
            nc.scalar.activation(
                sq[:], xh[s // sph][:, SQW * (s % sph):SQW * (s % sph + 1)],
                AF.Square)
            sqs.append(sq)
        tps = SQW // C  # tiles per square slice
        for t in range(NT):
            ot = opool.tile([P, C], FP32, tag="ot")
            sq = sqs[t // tps]
            off = C * (t % tps)
            nc.vector.scalar_tensor_tensor(ot[:], att[:], sig[:, t:t + 1],
                                           sq[:, off:off + C],
                                           op0=ALU.add, op1=ALU.mult)
            nc.sync.dma_start(out_d[b, P * t:P * (t + 1), :], ot[:])


def _build_nc(reps=1):
    nc = bacc.Bacc("TRN2", target_bir_lowering=False, debug=False,
                   enable_asserts=False, num_devices=N_CORES)
    x_d = nc.dram_tensor("xb", [NB, L, C], FP32, kind="ExternalInput").ap()
    w1_d = nc.dram_tensor("w1sb", [P, 2 * (HID + 1)], FP32, kind="ExternalInput").ap()
    b1_d = nc.dram_tensor("b1col", [HID + 1, 1], FP32, kind="ExternalInput").ap()
    w2b_d = nc.dram_tensor("w2b", [HID + 1, C], FP32, kind="ExternalInput").ap()
    cm_d = nc.dram_tensor("convmain", [P, 2 * P], FP32, kind="ExternalInput").ap()
    cc_d = nc.dram_tensor("convcorner", [P, 4 * P], FP32, kind="ExternalInput").ap()
    ones_d = nc.dram_tensor("ones", [P, P], FP32, kind="ExternalInput").ap()
    id_d = nc.dram_tensor("ident", [P, P], FP32, kind="ExternalInput").ap()
    rc_d = nc.dram_tensor("redcol", [P, 1], FP32, kind="ExternalInput").ap()
    out_d = nc.dram_tensor("out", [NB, L, C], FP32, kind="ExternalOutput").ap()

    with tile.TileContext(nc) as tc:
        with ExitStack() as ctx:
            _build_body(ctx, tc, out_d, x_d, w1_d, b1_d, w2b_d, cm_d, cc_d,
                        ones_d, id_d, rc_d, reps=reps)
    nc.compile()
    return nc


def get_nc(reps=1):
    key = f"nc{reps}"
    if key not in _CACHE:
        _CACHE[key] = _build_nc(reps=reps)
    return _CACHE[key]


def _prep_inputs(W1, b1, W2, b2, conv_w):
    """Host-side parameter preprocessing (shared across cores)."""
    W1 = np.asarray(W1, np.float32)
    W2 = np.asarray(W2, np.float32)
    b1 = np.asarray(b1, np.float32)
    b2 = np.asarray(b2, np.float32)
    conv_w = np.asarray(conv_w, np.float32)

    HB = HID + 1
    w1sb = np.zeros((P, 2 * HB), np.float32)
    for h in range(2):
        w1sb[:, HB * h:HB * h + HID] = W1[P * h:P * (h + 1), :]
    w2b = np.concatenate([W2, b2[None, :]], axis=0).astype(np.float32)
    b1col = np.concatenate([b1, [1.0]]).astype(np.float32).reshape(HB, 1)

    # Banded Toeplitz over two adjacent 128-blocks; avg band folds in the
    # 1/C spatial-mean scale (device computes raw channel sums).
    wa = conv_w[:, 0, 0] / C
    wm = conv_w[:, 1, 0]
    Wb_a = np.zeros((2 * P, 2 * P), np.float32)
    Wb_m = np.zeros((2 * P, 2 * P), np.float32)
    for i in range(2 * P):
        for k in range(7):
            j = i + k - 3
            if 0 <= j < 2 * P:
                Wb_a[i, j] = wa[k]
                Wb_m[i, j] = wm[k]
    cmain = np.concatenate([Wb_a[0:P, 0:P].T, Wb_m[0:P, 0:P].T], axis=1)
    # Corner lhsTs in one [128, 512] tensor. The prev-block ("lo") bands use
    # full K=128 (only rows 125-127 nonzero) so the rhs stays at base
    # partition 0 (PE requires base partition in {0, 32, 64}); the
    # next-block ("hi") bands are K=3 at rows 0-2.
    corn = np.zeros((P, 4 * P), np.float32)
    corn[:, 0:P] = Wb_a[P:2 * P, 0:P].T            # prev-block avg
    corn[:, P:2 * P] = Wb_m[P:2 * P, 0:P].T        # prev-block max
    corn[0:3, 2 * P:3 * P] = Wb_a[0:P, P:2 * P].T[0:3, :]   # next-block avg
    corn[0:3, 3 * P:4 * P] = Wb_m[0:P, P:2 * P].T[0:3, :]   # next-block max
    return {
        "w1sb": w1sb,
        "b1col": np.ascontiguousarray(b1col),
        "w2b": w2b,
        "convmain": np.ascontiguousarray(cmain),
        "convcorner": np.ascontiguousarray(corn),
        "ones": np.ones((P, P), np.float32),
        "ident": np.eye(P, dtype=np.float32),
        "redcol": np.full((P, 1), 1.0 / L, np.float32),
    }


def kernel(x, W1, b1, W2, b2, conv_w):
    nc = get_nc()
    x = np.asarray(x, np.float32)
    params = _prep_inputs(W1, b1, W2, b2, conv_w)
    in_maps = []
    for c in range(N_CORES):
        m = dict(params)
        m["xb"] = np.ascontiguousarray(x[NB * c:NB * (c + 1)])
        in_maps.append(m)
    _CACHE["last_in_maps"] = in_maps
    res = run_bass_kernel_spmd(nc, in_maps, list(range(N_CORES)))
    _CACHE["last_results"] = res
    return np.concatenate([res.results[c]["out"] for c in range(N_CORES)],
                          axis=0)


def _pjrt_exec(nc, in_maps, n_warm=2, n_time=8):
    """Build a sharded jit for nc, run it, return (best_wall_s, result)."""
    import time
    import jax
    import concourse.mybir as mybir_
    from concourse.bass2jax import (_bass_exec_p, install_neuronx_cc_hook,
                                    partition_id_tensor)
    from jax.experimental.shard_map import shard_map
    from jax.sharding import Mesh, PartitionSpec

    install_neuronx_cc_hook()
    partition_name = (nc.partition_id_tensor.name
                      if nc.partition_id_tensor else None)
    in_names, out_names, out_avals = [], [], []
    for alloc in nc.m.functions[0].allocations:
        if not isinstance(alloc, mybir_.MemoryLocationSet):
            continue
        name = alloc.memorylocations[0].name
        if alloc.kind == "ExternalInput":
            if name != partition_name:
                in_names.append(name)
        elif alloc.kind == "ExternalOutput":
            out_names.append(name)
            out_avals.append(jax.core.ShapedArray(
                tuple(alloc.tensor_shape), mybir_.dt.np(alloc.dtype)))
    n_params = len(in_names)
    all_in_names = list(in_names) + list(out_names)
    if partition_name is not None:
        all_in_names.append(partition_name)

    def _body(*args):
        operands = list(args)
        if partition_name is not None:
            operands.append(partition_id_tensor())
        return tuple(_bass_exec_p.bind(
            *operands,
            out_avals=tuple(out_avals),
            in_names=tuple(all_in_names),
            out_names=tuple(out_names),
            lowering_input_output_aliases=(),
            sim_require_finite=True,
            sim_require_nnan=True,
            nc=nc,
        ))

    devices = jax.devices()[:N_CORES]
    mesh = Mesh(np.asarray(devices), ("core",))
    nin = n_params + len(out_names)
    sharding = jax.sharding.NamedSharding(mesh, PartitionSpec("core"))
    fn = jax.jit(shard_map(
        _body, mesh=mesh,
        in_specs=(PartitionSpec("core"),) * nin,
        out_specs=(PartitionSpec("core"),) * len(out_names),
        check_rep=False))
    dev_args = [
        jax.device_put(np.concatenate(
            [np.asarray(in_maps[c][nm]) for c in range(N_CORES)], axis=0),
            sharding)
        for nm in in_names
    ]
    for av in out_avals:
        z = np.zeros((N_CORES * av.shape[0], *av.shape[1:]), av.dtype)
        dev_args.append(jax.device_put(z, sharding))

    for _ in range(n_warm):
        out = fn(*dev_args)
        jax.block_until_ready(out)
    best = float("inf")
    for _ in range(n_time):
        t0 = time.perf_counter()
        out = fn(*dev_args)
        jax.block_until_ready(out)
        best = min(best, time.perf_counter() - t0)
    result = np.asarray(out[0]).reshape(N_CORES * NB, L, C)
    return best, result


def bench_repeat(reps=8, n_time=10, in_maps=None):
    """Isolate device exec time: time a module doing the work `reps` times
    in-kernel vs once; slope = steady-state HW time per execution."""
    if in_maps is None:
        in_maps = _CACHE["last_in_maps"]
    t1, _ = _pjrt_exec(get_nc(1), in_maps, n_time=n_time)
    tr, result = _pjrt_exec(get_nc(reps), in_maps, n_time=n_time)
    per_exec_ns = (tr - t1) / (reps - 1) * 1e9
    return per_exec_ns, result, t1 * 1e9, tr * 1e9


def bench(n_iters=30, in_maps=None):
    """Time back-to-back NEFF executions with device-resident inputs.

    Mirrors bass2jax.run_bass_via_pjrt's multi-core path but without buffer
    donation so inputs (incl. zero-filled output buffers) stay reusable
    across iterations; reports amortized per-iteration wall time, which
    bounds true HW exec time from above by the per-dispatch overhead.
    """
    import time
    import jax
    import concourse.mybir as mybir_
    from concourse.bass2jax import (_bass_exec_p, install_neuronx_cc_hook,
                                    partition_id_tensor)
    from jax.experimental.shard_map import shard_map
    from jax.sharding import Mesh, PartitionSpec

    nc = get_nc()
    if in_maps is None:
        in_maps = _CACHE["last_in_maps"]
    install_neuronx_cc_hook()

    partition_name = (nc.partition_id_tensor.name
                      if nc.partition_id_tensor else None)
    in_names, out_names, out_avals, zero_outs = [], [], [], []
    for alloc in nc.m.functions[0].allocations:
        if not isinstance(alloc, mybir_.MemoryLocationSet):
            continue
        name = alloc.memorylocations[0].name
        if alloc.kind == "ExternalInput":
            if name != partition_name:
                in_names.append(name)
        elif alloc.kind == "ExternalOutput":
            shape = tuple(alloc.tensor_shape)
            dtype = mybir_.dt.np(alloc.dtype)
            out_names.append(name)
            out_avals.append(jax.core.ShapedArray(shape, dtype))
            zero_outs.append(np.zeros(shape, dtype))
    n_params = len(in_names)
    all_in_names = list(in_names) + list(out_names)
    if partition_name is not None:
        all_in_names.append(partition_name)

    def _body(*args):
        operands = list(args)
        if partition_name is not None:
            operands.append(partition_id_tensor())
        return tuple(_bass_exec_p.bind(
            *operands,
            out_avals=tuple(out_avals),
            in_names=tuple(all_in_names),
            out_names=tuple(out_names),
            lowering_input_output_aliases=(),
            sim_require_finite=True,
            sim_require_nnan=True,
            nc=nc,
        ))

    devices = jax.devices()[:N_CORES]
    mesh = Mesh(np.asarray(devices), ("core",))
    nin = n_params + len(out_names)
    sharded = jax.jit(shard_map(
        _body, mesh=mesh,
        in_specs=(PartitionSpec("core"),) * nin,
        out_specs=(PartitionSpec("core"),) * len(out_names),
        check_rep=False))

    concat_in = [
        np.concatenate([np.asarray(in_maps[c][nm]) for c in range(N_CORES)],
                       axis=0)
        for nm in in_names
    ]
    concat_zeros = [
        np.zeros((N_CORES * z.shape[0], *z.shape[1:]), z.dtype)
        for z in zero_outs
    ]
    sharding = jax.sharding.NamedSharding(mesh, PartitionSpec("core"))
    dev_args = [jax.device_put(a, sharding) for a in concat_in + concat_zeros]

    out = sharded(*dev_args)
    jax.block_until_ready(out)
    t0 = time.perf_counter()
    for _ in range(n_iters):
        out = sharded(*dev_args)
    jax.block_until_ready(out)
    t1 = time.perf_counter()
    per_iter_ns = (t1 - t0) / n_iters * 1e9
    result = np.asarray(out[0]).reshape(N_CORES * NB, L, C)
    return per_iter_ns, result



# revision 7
# speedup vs baseline: 1.4477x; 1.4477x over previous
"""CBAM kernel for Trainium2, 8-way batch-parallel SPMD, f16 data path.

Computes out = x^2 * (att_c[b,c] + sigmoid(conv(spatial_stats))[b,l]) where
att_c = sigmoid(mlp(mean_L x) + mlp(max_L x)), matching the CBAM reference.

Key layout decision: x is staged host-side as float16 in the SBUF tile
layout [NB, 128, NT*C] (partition = l % 128, free col = 256*(l//128) + c),
so each batch loads/stores as ONE dma_start of 128 x 16KB contiguous rows.
This halves HBM traffic vs f32 and collapses ~260 small DMAs into 8 big
ones (the f32 baseline was bottlenecked on per-DMA sequencing overhead,
sync engine 63% busy).

Engine split per batch:
  PE   : channel-sum (x-slices as rhs vs stationary 1/L column), PE
         transposes for channel-major stats, MLP, conv (banded-Toeplitz)
  DVE  : spatial sum+max (3D reduces, f16 4x mode), chan-max fold tree,
         half the final (att + sig) * x^2 scalar_tensor_tensor ops
  ACT  : x^2 squares, sigmoids/relu, psum->sbuf stat copies
  POOL : other half of the final stt ops, output store DMAs
"""

import numpy as np
from contextlib import ExitStack

import concourse.bacc as bacc
import concourse.bass as bass
import concourse.tile as tile
import concourse.mybir as mybir
from concourse.bass_utils import run_bass_kernel_spmd

AF = mybir.ActivationFunctionType
ALU = mybir.AluOpType
AX = mybir.AxisListType
FP32 = mybir.dt.float32
FP16 = mybir.dt.float16

N_CORES = 8
B_FULL = 32
NB = B_FULL // N_CORES  # batches per core = 4
L = 4096
C = 256
HID = 16
P = 128
NT = L // P  # 32 L-tiles per batch
F = NT * C   # 8192 free columns per batch

_CACHE: dict = {}


def _build_body(ctx: ExitStack, tc, out_d, x_d, w1_d, b1_d, w2b_d, cm_d, cc_d,
                ones_d, id_d, rc_d, reps=1):
    nc = tc.nc

    const = ctx.enter_context(tc.tile_pool(name="const", bufs=1))
    xpool = ctx.enter_context(tc.tile_pool(name="x", bufs=3))
    sqpool = ctx.enter_context(tc.tile_pool(name="sq", bufs=2))
    opool = ctx.enter_context(tc.tile_pool(name="outt", bufs=2))
    mpool = ctx.enter_context(tc.tile_pool(name="maxtree", bufs=2))
    spool = ctx.enter_context(tc.tile_pool(name="stats", bufs=2))
    apool = ctx.enter_context(tc.tile_pool(name="att", bufs=2))
    pacc = ctx.enter_context(tc.tile_pool(name="pacc", bufs=2, space="PSUM"))
    ptrp = ctx.enter_context(tc.tile_pool(name="ptrp", bufs=2, space="PSUM"))
    pwork = ctx.enter_context(tc.tile_pool(name="pwork", bufs=4, space="PSUM"))

    w1 = const.tile([P, 2 * (HID + 1)], FP32)
    nc.sync.dma_start(w1[:], w1_d[:])
    b1 = const.tile([HID + 1, 1], FP32)
    nc.sync.dma_start(b1[:], b1_d[:])
    w2b = const.tile([HID + 1, C], FP32)
    nc.sync.dma_start(w2b[:], w2b_d[:])
    cmain = const.tile([P, 2 * P], FP16)
    nc.sync.dma_start(cmain[:], cm_d[:])
    ccorn = const.tile([P, 4 * P], FP16)
    nc.sync.dma_start(ccorn[:], cc_d[:])
    ones = const.tile([P, P], FP32)
    nc.sync.dma_start(ones[:], ones_d[:])
    ident = const.tile([P, P], FP16)
    nc.sync.dma_start(ident[:], id_d[:])
    redcol = const.tile([P, 1], FP16)
    nc.sync.dma_start(redcol[:], rc_d[:])

    for b in [b for _ in range(reps) for b in range(NB)]:
        xt = xpool.tile([P, F], FP16, tag="x", name=f"x{b}")
        nc.sync.dma_start(xt[:], x_d[b])

        # ---- x^2 on ACT, two halves, emitted first so the final combine
        # isn't stuck behind the att/sig chain on the ACT queue ----
        sq = sqpool.tile([P, F], FP16, tag="sq")
        for h in range(2):
            nc.scalar.activation(sq[:, F // 2 * h:F // 2 * (h + 1)],
                                 xt[:, F // 2 * h:F // 2 * (h + 1)], AF.Square)

        # ---- channel mean over L (PE): stationary 1/L column, x streams
        # as rhs; pcs[0, c] accumulates mean over L in psum f32 ----
        pcs = pacc.tile([1, C], FP32, tag="pcs")
        for t in range(NT):
            nc.tensor.matmul(pcs[:], redcol[:], xt[:, C * t:C * (t + 1)],
                             start=(t == 0), stop=(t == NT - 1),
                             skip_group_check=True)

        # ---- chan-max fold tree (DVE, f16): mb[p, c] = max over t ----
        mb = mpool.tile([P, F // 2], FP16, tag="mb")
        nc.vector.tensor_max(mb[:], xt[:, 0:F // 2], xt[:, F // 2:F])
        w = F // 4
        while w >= C:
            nc.vector.tensor_max(mb[:, 0:w], mb[:, 0:w], mb[:, w:2 * w])
            w //= 2

        # ---- spatial max + sum over C: single 3D reduces (DVE) ----
        max_s = spool.tile([P, NT], FP16, tag="max_s")
        nc.vector.tensor_reduce(
            max_s[:], xt[:].rearrange("p (t c) -> p t c", c=C),
            axis=AX.X, op=ALU.max)
        sum_s = spool.tile([P, NT], FP16, tag="sum_s")
        with nc.allow_low_precision(reason="f16 spatial sum feeds sigmoid"):
            nc.vector.tensor_reduce(
                sum_s[:], xt[:].rearrange("p (t c) -> p t c", c=C),
                axis=AX.X, op=ALU.add)

        # ---- channel-major stats via PE transposes ----
        # avg: pcs [1, C] -> avg_row f16 -> two [1,128] transposes -> [128, 2]
        avg_row = spool.tile([1, C], FP16, tag="avg_row")
        nc.scalar.activation(avg_row[:], pcs[:], AF.Copy)
        pavgT = pwork.tile([P, 4], FP16, tag="pwork")
        for h in range(2):
            nc.tensor.transpose(pavgT[:, 2 * h:2 * h + 1],
                                avg_row[0:1, P * h:P * (h + 1)],
                                ident[0:1, 0:1])
        # max: mb[:, 0:C] halves -> [128, 256] transposed, reduce over p
        pmaxT = ptrp.tile([P, 2 * P], FP16, tag="pmaxT")
        for h in range(2):
            nc.tensor.transpose(pmaxT[:, P * h:P * (h + 1)],
                                mb[:, P * h:P * (h + 1)], ident[:])
        # stats_cm[c % 128, 2h + {0,1}] = {avg, max} of channel 128h + c%128
        stats_cm = spool.tile([P, 4], FP32, tag="stats_cm")
        for h in range(2):
            nc.scalar.activation(stats_cm[:, 2 * h:2 * h + 1],
                                 pavgT[:, 2 * h:2 * h + 1], AF.Copy)
            nc.vector.tensor_reduce(stats_cm[:, 2 * h + 1:2 * h + 2],
                                    pmaxT[:, P * h:P * (h + 1)],
                                    axis=AX.X, op=ALU.max)

        # ---- shared MLP (f32, tiny): row HID carries the 2*b2 constant ----
        ph = pwork.tile([HID + 1, 2], FP32, tag="pwork")
        nc.tensor.matmul(ph[:], w1[:, 0:HID + 1], stats_cm[:, 0:2],
                         start=True, stop=False, skip_group_check=True)
        nc.tensor.matmul(ph[:], w1[:, HID + 1:2 * (HID + 1)], stats_cm[:, 2:4],
                         start=False, stop=True, skip_group_check=True)
        hsb = spool.tile([HID + 1, 2], FP32, tag="hsb")
        nc.scalar.activation(hsb[:], ph[:], AF.Relu, bias=b1[:])
        h2 = spool.tile([HID + 1, 1], FP32, tag="h2")
        nc.vector.tensor_add(h2[:], hsb[:, 0:1], hsb[:, 1:2])
        h2r = spool.tile([HID + 1, P], FP32, tag="h2r")
        nc.vector.tensor_scalar_mul(h2r[:], ones[0:HID + 1, :], h2[:])
        po = pwork.tile([P, C], FP32, tag="pwork")
        nc.tensor.matmul(po[:], h2r[:], w2b[:], start=True, stop=True,
                         skip_group_check=True)
        att = apool.tile([P, C], FP16, tag="att")
        nc.scalar.activation(att[:], po[:], AF.Sigmoid)

        # ---- spatial conv over L: banded-Toeplitz matmuls (f16 in) ----
        pc = pwork.tile([P, NT], FP32, tag="pwork")
        nc.tensor.matmul(pc[:, :], cmain[:, 0:P], sum_s[:],
                         start=True, stop=False, skip_group_check=True)
        nc.tensor.matmul(pc[:, :], cmain[:, P:2 * P], max_s[:],
                         start=False, stop=False, skip_group_check=True)
        nc.tensor.matmul(pc[:, 1:NT], ccorn[:, 0:P], sum_s[:, 0:NT - 1],
                         start=False, stop=False, skip_group_check=True)
        nc.tensor.matmul(pc[:, 1:NT], ccorn[:, P:2 * P],
                         max_s[:, 0:NT - 1],
                         start=False, stop=False, skip_group_check=True)
        nc.tensor.matmul(pc[:, 0:NT - 1], ccorn[0:3, 2 * P:3 * P],
                         sum_s[0:3, 1:NT],
                         start=False, stop=False, skip_group_check=True)
        nc.tensor.matmul(pc[:, 0:NT - 1], ccorn[0:3, 3 * P:4 * P],
                         max_s[0:3, 1:NT],
                         start=False, stop=True, skip_group_check=True)
        sig = apool.tile([P, NT], FP16, tag="sig")
        nc.scalar.activation(sig[:], pc[:], AF.Sigmoid)

        # ---- final: out = (att + sig) * x^2, split DVE/Pool ----
        ot = opool.tile([P, F], FP16, tag="ot")
        for t in range(NT):
            nc.vector.scalar_tensor_tensor(ot[:, C * t:C * (t + 1)], att[:],
                                           sig[:, t:t + 1],
                                           sq[:, C * t:C * (t + 1)],
                                           op0=ALU.add, op1=ALU.mult)
        nc.gpsimd.dma_start(out_d[b], ot[:])


def _build_nc(reps=1):
    nc = bacc.Bacc("TRN2", target_bir_lowering=False, debug=False,
                   enable_asserts=False, num_devices=N_CORES)
    x_d = nc.dram_tensor("xb", [NB, P, F], FP16, kind="ExternalInput").ap()
    w1_d = nc.dram_tensor("w1sb", [P, 2 * (HID + 1)], FP32, kind="ExternalInput").ap()
    b1_d = nc.dram_tensor("b1col", [HID + 1, 1], FP32, kind="ExternalInput").ap()
    w2b_d = nc.dram_tensor("w2b", [HID + 1, C], FP32, kind="ExternalInput").ap()
    cm_d = nc.dram_tensor("convmain", [P, 2 * P], FP16, kind="ExternalInput").ap()
    cc_d = nc.dram_tensor("convcorner", [P, 4 * P], FP16, kind="ExternalInput").ap()
    ones_d = nc.dram_tensor("ones", [P, P], FP32, kind="ExternalInput").ap()
    id_d = nc.dram_tensor("ident", [P, P], FP16, kind="ExternalInput").ap()
    rc_d = nc.dram_tensor("redcol", [P, 1], FP16, kind="ExternalInput").ap()
    out_d = nc.dram_tensor("out", [NB, P, F], FP16, kind="ExternalOutput").ap()

    with tile.TileContext(nc) as tc:
        with ExitStack() as ctx:
            _build_body(ctx, tc, out_d, x_d, w1_d, b1_d, w2b_d, cm_d, cc_d,
                        ones_d, id_d, rc_d, reps=reps)
    nc.compile()
    return nc


def get_nc(reps=1):
    key = f"nc{reps}"
    if key not in _CACHE:
        _CACHE[key] = _build_nc(reps=reps)
    return _CACHE[key]


def _prep_inputs(W1, b1, W2, b2, conv_w):
    """Host-side parameter preprocessing (shared across cores)."""
    W1 = np.asarray(W1, np.float32)
    W2 = np.asarray(W2, np.float32)
    b1 = np.asarray(b1, np.float32)
    b2 = np.asarray(b2, np.float32)
    conv_w = np.asarray(conv_w, np.float32)

    HB = HID + 1
    w1sb = np.zeros((P, 2 * HB), np.float32)
    for h in range(2):
        w1sb[:, HB * h:HB * h + HID] = W1[P * h:P * (h + 1), :]
    w2b = np.concatenate([W2, b2[None, :]], axis=0).astype(np.float32)
    b1col = np.concatenate([b1, [1.0]]).astype(np.float32).reshape(HB, 1)

    # Banded Toeplitz over two adjacent 128-blocks; avg band folds in the
    # 1/C spatial-mean scale (device computes raw channel sums).
    wa = conv_w[:, 0, 0] / C
    wm = conv_w[:, 1, 0]
    Wb_a = np.zeros((2 * P, 2 * P), np.float32)
    Wb_m = np.zeros((2 * P, 2 * P), np.float32)
    for i in range(2 * P):
        for k in range(7):
            j = i + k - 3
            if 0 <= j < 2 * P:
                Wb_a[i, j] = wa[k]
                Wb_m[i, j] = wm[k]
    cmain = np.concatenate([Wb_a[0:P, 0:P].T, Wb_m[0:P, 0:P].T], axis=1)
    # Corner lhsTs in one [128, 512] tensor. The prev-block ("lo") bands use
    # full K=128 (only rows 125-127 nonzero) so the rhs stays at base
    # partition 0 (PE requires base partition in {0, 32, 64}); the
    # next-block ("hi") bands are K=3 at rows 0-2.
    corn = np.zeros((P, 4 * P), np.float32)
    corn[:, 0:P] = Wb_a[P:2 * P, 0:P].T            # prev-block avg
    corn[:, P:2 * P] = Wb_m[P:2 * P, 0:P].T        # prev-block max
    corn[0:3, 2 * P:3 * P] = Wb_a[0:P, P:2 * P].T[0:3, :]   # next-block avg
    corn[0:3, 3 * P:4 * P] = Wb_m[0:P, P:2 * P].T[0:3, :]   # next-block max
    return {
        "w1sb": w1sb,
        "b1col": np.ascontiguousarray(b1col),
        "w2b": w2b,
        "convmain": np.ascontiguousarray(cmain).astype(np.float16),
        "convcorner": np.ascontiguousarray(corn).astype(np.float16),
        "ones": np.ones((P, P), np.float32),
        "ident": np.eye(P, dtype=np.float16),
        "redcol": np.full((P, 1), 1.0 / L, np.float16),
    }


def kernel(x, W1, b1, W2, b2, conv_w):
    nc = get_nc()
    x = np.asarray(x, np.float32)
    params = _prep_inputs(W1, b1, W2, b2, conv_w)
    # Stage x as f16 in the SBUF tile layout: [NB, 128, NT*C] with
    # col = 256 * (l // 128) + c, partition = l % 128.
    xt = x.reshape(B_FULL, NT, P, C).transpose(0, 2, 1, 3).reshape(
        B_FULL, P, F).astype(np.float16)
    in_maps = []
    for c in range(N_CORES):
        m = dict(params)
        m["xb"] = np.ascontiguousarray(xt[NB * c:NB * (c + 1)])
        in_maps.append(m)
    _CACHE["last_in_maps"] = in_maps
    res = run_bass_kernel_spmd(nc, in_maps, list(range(N_CORES)))
    _CACHE["last_results"] = res
    out = np.concatenate([res.results[c]["out"] for c in range(N_CORES)],
                         axis=0)
    # [B, 128, NT*C] f16 -> [B, L, C] f32
    return out.reshape(B_FULL, P, NT, C).transpose(0, 2, 1, 3).reshape(
        B_FULL, L, C).astype(np.float32)


def _pjrt_exec(nc, in_maps, n_warm=2, n_time=8):
    """Build a sharded jit for nc, run it, return (best_wall_s, result)."""
    import time
    import jax
    import concourse.mybir as mybir_
    from concourse.bass2jax import (_bass_exec_p, install_neuronx_cc_hook,
                                    partition_id_tensor)
    from jax.experimental.shard_map import shard_map
    from jax.sharding import Mesh, PartitionSpec

    install_neuronx_cc_hook()
    partition_name = (nc.partition_id_tensor.name
                      if nc.partition_id_tensor else None)
    in_names, out_names, out_avals = [], [], []
    for alloc in nc.m.functions[0].allocations:
        if not isinstance(alloc, mybir_.MemoryLocationSet):
            continue
        name = alloc.memorylocations[0].name
        if alloc.kind == "ExternalInput":
            if name != partition_name:
                in_names.append(name)
        elif alloc.kind == "ExternalOutput":
            out_names.append(name)
            out_avals.append(jax.core.ShapedArray(
                tuple(alloc.tensor_shape), mybir_.dt.np(alloc.dtype)))
    n_params = len(in_names)
    all_in_names = list(in_names) + list(out_names)
    if partition_name is not None:
        all_in_names.append(partition_name)

    def _body(*args):
        operands = list(args)
        if partition_name is not None:
            operands.append(partition_id_tensor())
        return tuple(_bass_exec_p.bind(
            *operands,
            out_avals=tuple(out_avals),
            in_names=tuple(all_in_names),
            out_names=tuple(out_names),
            lowering_input_output_aliases=(),
            sim_require_finite=True,
            sim_require_nnan=True,
            nc=nc,
        ))

    devices = jax.devices()[:N_CORES]
    mesh = Mesh(np.asarray(devices), ("core",))
    nin = n_params + len(out_names)
    sharding = jax.sharding.NamedSharding(mesh, PartitionSpec("core"))
    fn = jax.jit(shard_map(
        _body, mesh=mesh,
        in_specs=(PartitionSpec("core"),) * nin,
        out_specs=(PartitionSpec("core"),) * len(out_names),
        check_rep=False))
    dev_args = [
        jax.device_put(np.concatenate(
            [np.asarray(in_maps[c][nm]) for c in range(N_CORES)], axis=0),
            sharding)
        for nm in in_names
    ]
    for av in out_avals:
        z = np.zeros((N_CORES * av.shape[0], *av.shape[1:]), av.dtype)
        dev_args.append(jax.device_put(z, sharding))

    for _ in range(n_warm):
        out = fn(*dev_args)
        jax.block_until_ready(out)
    best = float("inf")
    for _ in range(n_time):
        t0 = time.perf_counter()
        out = fn(*dev_args)
        jax.block_until_ready(out)
        best = min(best, time.perf_counter() - t0)
    result = np.asarray(out[0])
    return best, result


def bench_repeat(reps=8, n_time=10, in_maps=None):
    """Isolate device exec time: time a module doing the work `reps` times
    in-kernel vs once; slope = steady-state HW time per execution."""
    if in_maps is None:
        in_maps = _CACHE["last_in_maps"]
    t1, _ = _pjrt_exec(get_nc(1), in_maps, n_time=n_time)
    tr, result = _pjrt_exec(get_nc(reps), in_maps, n_time=n_time)
    per_exec_ns = (tr - t1) / (reps - 1) * 1e9
    return per_exec_ns, result, t1 * 1e9, tr * 1e9


# revision 9
# speedup vs baseline: 1.8243x; 1.2602x over previous
"""CBAM kernel for Trainium2, 8-way batch-parallel SPMD, f16 data path.

Computes out = x^2 * (att_c[b,c] + sigmoid(conv(spatial_stats))[b,l]) where
att_c = sigmoid(mlp(mean_L x) + mlp(max_L x)), matching the CBAM reference.

Key layout decision: x is staged host-side as float16 in the SBUF tile
layout [NB, 128, NT*C] (partition = l % 128, free col = 256*(l//128) + c),
so each batch loads/stores as ONE dma_start of 128 x 16KB contiguous rows.
This halves HBM traffic vs f32 and collapses ~260 small DMAs into 8 big
ones (the f32 baseline was bottlenecked on per-DMA sequencing overhead,
sync engine 63% busy).

Engine split per batch:
  PE   : channel-sum (x-slices as rhs vs stationary 1/L column), PE
         transposes for channel-major stats, MLP, conv (banded-Toeplitz)
  DVE  : spatial sum+max (3D reduces, f16 4x mode), chan-max fold tree,
         half the final (att + sig) * x^2 scalar_tensor_tensor ops
  ACT  : x^2 squares, sigmoids/relu, psum->sbuf stat copies
  POOL : other half of the final stt ops, output store DMAs
"""

import numpy as np
from contextlib import ExitStack

import concourse.bacc as bacc
import concourse.bass as bass
import concourse.tile as tile
import concourse.mybir as mybir
from concourse.bass_utils import run_bass_kernel_spmd

AF = mybir.ActivationFunctionType
ALU = mybir.AluOpType
AX = mybir.AxisListType
FP32 = mybir.dt.float32
FP16 = mybir.dt.float16

N_CORES = 8
B_FULL = 32
NB = B_FULL // N_CORES  # batches per core = 4
L = 4096
C = 256
HID = 16
P = 128
NT = L // P  # 32 L-tiles per batch
F = NT * C   # 8192 free columns per batch

_CACHE: dict = {}


SATT_DVE = 8      # tiles whose att+sig runs on DVE tensor_scalar (rest ACT)
MUL_DVE = 5632    # columns of the final multiply on DVE (rest Pool)


def _build_body(ctx: ExitStack, tc, out_d, x_d, w1_d, b1_d, w2b_d, cm_d, cc_d,
                ones_d, id_d, rc_d, reps=1):
    nc = tc.nc

    const = ctx.enter_context(tc.tile_pool(name="const", bufs=1))
    xpool = ctx.enter_context(tc.tile_pool(name="x", bufs=2))
    sqpool = ctx.enter_context(tc.tile_pool(name="sq", bufs=2))
    stpool = ctx.enter_context(tc.tile_pool(name="satt", bufs=2))
    opool = ctx.enter_context(tc.tile_pool(name="outt", bufs=2))
    mpool = ctx.enter_context(tc.tile_pool(name="maxtree", bufs=2))
    spool = ctx.enter_context(tc.tile_pool(name="stats", bufs=2))
    apool = ctx.enter_context(tc.tile_pool(name="att", bufs=2))
    pacc = ctx.enter_context(tc.tile_pool(name="pacc", bufs=2, space="PSUM"))
    ptrp = ctx.enter_context(tc.tile_pool(name="ptrp", bufs=2, space="PSUM"))
    pwork = ctx.enter_context(tc.tile_pool(name="pwork", bufs=4, space="PSUM"))

    w1 = const.tile([P, 2 * (HID + 1)], FP32)
    nc.sync.dma_start(w1[:], w1_d[:])
    b1 = const.tile([HID + 1, 1], FP32)
    nc.sync.dma_start(b1[:], b1_d[:])
    w2b = const.tile([HID + 1, C], FP32)
    nc.sync.dma_start(w2b[:], w2b_d[:])
    cmain = const.tile([P, 2 * P], FP16)
    nc.sync.dma_start(cmain[:], cm_d[:])
    ccorn = const.tile([P, 4 * P], FP16)
    nc.sync.dma_start(ccorn[:], cc_d[:])
    ones = const.tile([P, P], FP32)
    nc.sync.dma_start(ones[:], ones_d[:])
    ident = const.tile([P, P], FP16)
    nc.sync.dma_start(ident[:], id_d[:])
    redcol = const.tile([P, 1], FP16)
    nc.sync.dma_start(redcol[:], rc_d[:])

    def spatial_tree(xt, fa, fb, out, op):
        """Per-(p, t) reduce over c via a 2x-mode TT fold tree + small tail.

        Folds c: 256 -> 8 through fa/fb ping-pong, then one cheap 3D reduce.
        """
        x4 = xt[:].rearrange("p (t h c) -> p t h c", h=2, c=128)
        nc.vector.tensor_tensor(
            fa[:, 0:4096].rearrange("p (t h c) -> p t h c", h=1, c=128),
            x4[:, :, 0:1, :], x4[:, :, 1:2, :], op=op)
        seq = [(fa, 4096), (fb, 2048), (fa, 1024), (fb, 512), (fa, 256)]
        for i in range(1, 5):
            sbuf, sw = seq[i - 1]
            dbuf, dw = seq[i]
            ch = sw // NT // 2
            s4 = sbuf[:, 0:sw].rearrange("p (t h c) -> p t h c", h=2, c=ch)
            nc.vector.tensor_tensor(
                dbuf[:, 0:dw].rearrange("p (t h c) -> p t h c", h=1, c=ch),
                s4[:, :, 0:1, :], s4[:, :, 1:2, :], op=op)
        with nc.allow_low_precision(reason="f16 spatial stats feed sigmoid"):
            nc.vector.tensor_reduce(
                out[:], fa[:, 0:256].rearrange("p (t c) -> p t c", c=8),
                axis=AX.X, op=op)

    def emit_stats(b):
        st = {}
        xt = xpool.tile([P, F], FP16, tag="x", name=f"x{b}")
        for h in range(2):
            nc.sync.dma_start(xt[:, F // 2 * h:F // 2 * (h + 1)],
                              x_d[b, :, F // 2 * h:F // 2 * (h + 1)])
        st["xt"] = xt

        # x^2 on ACT first: only depends on x, fills ACT while stats run
        sq = sqpool.tile([P, F], FP16, tag="sq")
        for h in range(2):
            nc.scalar.activation(sq[:, F // 2 * h:F // 2 * (h + 1)],
                                 xt[:, F // 2 * h:F // 2 * (h + 1)], AF.Square)
        st["sq"] = sq

        # channel sum over L (PE): 16 matmuls of [1, 512]; even tiles land
        # in cols 0:256, odd tiles in 256:512 (folded by DVE below)
        pcs2 = pacc.tile([1, 2 * C], FP32, tag="pcs")
        for j in range(NT // 2):
            nc.tensor.matmul(pcs2[:], redcol[:], xt[:, 2 * C * j:2 * C * (j + 1)],
                             start=(j == 0), stop=(j == NT // 2 - 1),
                             skip_group_check=True)

        # chan-max fold tree (DVE, contiguous halves)
        mb = mpool.tile([P, F // 2], FP16, tag="mb")
        nc.vector.tensor_max(mb[:], xt[:, 0:F // 2], xt[:, F // 2:F])
        w = F // 4
        while w >= C:
            nc.vector.tensor_max(mb[:, 0:w], mb[:, 0:w], mb[:, w:2 * w])
            w //= 2
        # cross-partition max via PE transposes + small DVE reduces
        pmaxT = ptrp.tile([P, 2 * P], FP16, tag="pmaxT")
        for h in range(2):
            nc.tensor.transpose(pmaxT[:, P * h:P * (h + 1)],
                                mb[:, P * h:P * (h + 1)], ident[:])
        stats_cm = spool.tile([P, 4], FP32, tag="stats_cm")
        for h in range(2):
            nc.vector.tensor_reduce(stats_cm[:, 2 * h + 1:2 * h + 2],
                                    pmaxT[:, P * h:P * (h + 1)],
                                    axis=AX.X, op=ALU.max)

        # spatial max + sum trees (DVE)
        fa = mpool.tile([P, 4096], FP16, tag="fa")
        fb = mpool.tile([P, 2048], FP16, tag="fb")
        max_s = spool.tile([P, NT], FP16, tag="max_s")
        spatial_tree(xt, fa, fb, max_s, ALU.max)
        # avg fold: [1, 512] psum -> sbuf copy -> [1, 256] f16
        sr512 = spool.tile([1, 2 * C], FP16, tag="sr512")
        nc.scalar.activation(sr512[:], pcs2[:], AF.Copy)
        avg_row = spool.tile([1, C], FP16, tag="avg_row")
        nc.vector.tensor_add(avg_row[:], sr512[0:1, 0:C], sr512[0:1, C:2 * C])
        sum_s = spool.tile([P, NT], FP16, tag="sum_s")
        spatial_tree(xt, fa, fb, sum_s, ALU.add)

        # avg transposes into channel-major + ACT copies
        pavgT = pwork.tile([P, 4], FP16, tag="pwork")
        for h in range(2):
            nc.tensor.transpose(pavgT[:, 2 * h:2 * h + 1],
                                avg_row[0:1, P * h:P * (h + 1)],
                                ident[0:1, 0:1])
        for h in range(2):
            nc.scalar.activation(stats_cm[:, 2 * h:2 * h + 1],
                                 pavgT[:, 2 * h:2 * h + 1], AF.Copy)

        # shared MLP (f32, tiny): row HID carries the 2*b2 constant
        ph = pwork.tile([HID + 1, 2], FP32, tag="pwork")
        nc.tensor.matmul(ph[:], w1[:, 0:HID + 1], stats_cm[:, 0:2],
                         start=True, stop=False, skip_group_check=True)
        nc.tensor.matmul(ph[:], w1[:, HID + 1:2 * (HID + 1)], stats_cm[:, 2:4],
                         start=False, stop=True, skip_group_check=True)
        hsb = spool.tile([HID + 1, 2], FP32, tag="hsb")
        nc.scalar.activation(hsb[:], ph[:], AF.Relu, bias=b1[:])
        h2 = spool.tile([HID + 1, 1], FP32, tag="h2")
        nc.vector.tensor_add(h2[:], hsb[:, 0:1], hsb[:, 1:2])
        h2r = spool.tile([HID + 1, P], FP32, tag="h2r")
        nc.vector.tensor_scalar_mul(h2r[:], ones[0:HID + 1, :], h2[:])
        po = pwork.tile([P, C], FP32, tag="pwork")
        nc.tensor.matmul(po[:], h2r[:], w2b[:], start=True, stop=True,
                         skip_group_check=True)
        att = apool.tile([P, C], FP16, tag="att")
        nc.scalar.activation(att[:], po[:], AF.Sigmoid)
        st["att"] = att

        # spatial conv over L: banded-Toeplitz matmuls (f16 in)
        pc = pwork.tile([P, NT], FP32, tag="pwork")
        nc.tensor.matmul(pc[:, :], cmain[:, 0:P], sum_s[:],
                         start=True, stop=False, skip_group_check=True)
        nc.tensor.matmul(pc[:, :], cmain[:, P:2 * P], max_s[:],
                         start=False, stop=False, skip_group_check=True)
        nc.tensor.matmul(pc[:, 1:NT], ccorn[:, 0:P], sum_s[:, 0:NT - 1],
                         start=False, stop=False, skip_group_check=True)
        nc.tensor.matmul(pc[:, 1:NT], ccorn[:, P:2 * P], max_s[:, 0:NT - 1],
                         start=False, stop=False, skip_group_check=True)
        nc.tensor.matmul(pc[:, 0:NT - 1], ccorn[0:3, 2 * P:3 * P],
                         sum_s[0:3, 1:NT],
                         start=False, stop=False, skip_group_check=True)
        nc.tensor.matmul(pc[:, 0:NT - 1], ccorn[0:3, 3 * P:4 * P],
                         max_s[0:3, 1:NT],
                         start=False, stop=True, skip_group_check=True)
        sig = apool.tile([P, NT], FP32, tag="sig")
        nc.scalar.activation(sig[:], pc[:], AF.Sigmoid)
        st["sig"] = sig
        st["b"] = b
        return st

    def emit_final(st):
        att, sig, sq = st["att"], st["sig"], st["sq"]
        # satt[:, 256t + c] = att[c] + sig[p, t]: DVE tensor_scalar for the
        # first SATT_DVE tiles, ACT identity-with-bias for the rest
        satt = stpool.tile([P, F], FP16, tag="satt")
        for t in range(SATT_DVE):
            nc.vector.tensor_scalar_add(satt[:, C * t:C * (t + 1)], att[:],
                                        sig[:, t:t + 1])
        for t in range(SATT_DVE, NT):
            nc.scalar.activation(satt[:, C * t:C * (t + 1)], att[:],
                                 AF.Identity, bias=sig[:, t:t + 1])
        # out = satt * x^2: one big DVE TT + Pool TT tail
        ot = opool.tile([P, F], FP16, tag="ot")
        nc.vector.tensor_mul(ot[:, 0:MUL_DVE], satt[:, 0:MUL_DVE],
                             sq[:, 0:MUL_DVE])
        nc.gpsimd.tensor_tensor(ot[:, MUL_DVE:F], satt[:, MUL_DVE:F],
                                sq[:, MUL_DVE:F], op=ALU.mult)
        nc.gpsimd.dma_start(out_d[st["b"]], ot[:])

    # software-pipelined emission: final(b-1) lands after stats(b) so no
    # engine stalls head-of-line on the cross-engine satt join
    prev = None
    for b in [b for _ in range(reps) for b in range(NB)]:
        cur = emit_stats(b)
        if prev is not None:
            emit_final(prev)
        prev = cur
    emit_final(prev)


def _build_nc(reps=1):
    nc = bacc.Bacc("TRN2", target_bir_lowering=False, debug=False,
                   enable_asserts=False, num_devices=N_CORES)
    x_d = nc.dram_tensor("xb", [NB, P, F], FP16, kind="ExternalInput").ap()
    w1_d = nc.dram_tensor("w1sb", [P, 2 * (HID + 1)], FP32, kind="ExternalInput").ap()
    b1_d = nc.dram_tensor("b1col", [HID + 1, 1], FP32, kind="ExternalInput").ap()
    w2b_d = nc.dram_tensor("w2b", [HID + 1, C], FP32, kind="ExternalInput").ap()
    cm_d = nc.dram_tensor("convmain", [P, 2 * P], FP16, kind="ExternalInput").ap()
    cc_d = nc.dram_tensor("convcorner", [P, 4 * P], FP16, kind="ExternalInput").ap()
    ones_d = nc.dram_tensor("ones", [P, P], FP32, kind="ExternalInput").ap()
    id_d = nc.dram_tensor("ident", [P, P], FP16, kind="ExternalInput").ap()
    rc_d = nc.dram_tensor("redcol", [P, 1], FP16, kind="ExternalInput").ap()
    out_d = nc.dram_tensor("out", [NB, P, F], FP16, kind="ExternalOutput").ap()

    with tile.TileContext(nc) as tc:
        with ExitStack() as ctx:
            _build_body(ctx, tc, out_d, x_d, w1_d, b1_d, w2b_d, cm_d, cc_d,
                        ones_d, id_d, rc_d, reps=reps)
    nc.compile()
    return nc


def get_nc(reps=1):
    key = f"nc{reps}"
    if key not in _CACHE:
        _CACHE[key] = _build_nc(reps=reps)
    return _CACHE[key]


def _prep_inputs(W1, b1, W2, b2, conv_w):
    """Host-side parameter preprocessing (shared across cores)."""
    W1 = np.asarray(W1, np.float32)
    W2 = np.asarray(W2, np.float32)
    b1 = np.asarray(b1, np.float32)
    b2 = np.asarray(b2, np.float32)
    conv_w = np.asarray(conv_w, np.float32)

    HB = HID + 1
    w1sb = np.zeros((P, 2 * HB), np.float32)
    for h in range(2):
        w1sb[:, HB * h:HB * h + HID] = W1[P * h:P * (h + 1), :]
    w2b = np.concatenate([W2, b2[None, :]], axis=0).astype(np.float32)
    b1col = np.concatenate([b1, [1.0]]).astype(np.float32).reshape(HB, 1)

    # Banded Toeplitz over two adjacent 128-blocks; avg band folds in the
    # 1/C spatial-mean scale (device computes raw channel sums).
    wa = conv_w[:, 0, 0] / C
    wm = conv_w[:, 1, 0]
    Wb_a = np.zeros((2 * P, 2 * P), np.float32)
    Wb_m = np.zeros((2 * P, 2 * P), np.float32)
    for i in range(2 * P):
        for k in range(7):
            j = i + k - 3
            if 0 <= j < 2 * P:
                Wb_a[i, j] = wa[k]
                Wb_m[i, j] = wm[k]
    cmain = np.concatenate([Wb_a[0:P, 0:P].T, Wb_m[0:P, 0:P].T], axis=1)
    # Corner lhsTs in one [128, 512] tensor. The prev-block ("lo") bands use
    # full K=128 (only rows 125-127 nonzero) so the rhs stays at base
    # partition 0 (PE requires base partition in {0, 32, 64}); the
    # next-block ("hi") bands are K=3 at rows 0-2.
    corn = np.zeros((P, 4 * P), np.float32)
    corn[:, 0:P] = Wb_a[P:2 * P, 0:P].T            # prev-block avg
    corn[:, P:2 * P] = Wb_m[P:2 * P, 0:P].T        # prev-block max
    corn[0:3, 2 * P:3 * P] = Wb_a[0:P, P:2 * P].T[0:3, :]   # next-block avg
    corn[0:3, 3 * P:4 * P] = Wb_m[0:P, P:2 * P].T[0:3, :]   # next-block max
    return {
        "w1sb": w1sb,
        "b1col": np.ascontiguousarray(b1col),
        "w2b": w2b,
        "convmain": np.ascontiguousarray(cmain).astype(np.float16),
        "convcorner": np.ascontiguousarray(corn).astype(np.float16),
        "ones": np.ones((P, P), np.float32),
        "ident": np.eye(P, dtype=np.float16),
        "redcol": np.full((P, 1), 1.0 / L, np.float16),
    }


def kernel(x, W1, b1, W2, b2, conv_w):
    nc = get_nc()
    x = np.asarray(x, np.float32)
    params = _prep_inputs(W1, b1, W2, b2, conv_w)
    # Stage x as f16 in the SBUF tile layout: [NB, 128, NT*C] with
    # col = 256 * (l // 128) + c, partition = l % 128.
    xt = x.reshape(B_FULL, NT, P, C).transpose(0, 2, 1, 3).reshape(
        B_FULL, P, F).astype(np.float16)
    in_maps = []
    for c in range(N_CORES):
        m = dict(params)
        m["xb"] = np.ascontiguousarray(xt[NB * c:NB * (c + 1)])
        in_maps.append(m)
    _CACHE["last_in_maps"] = in_maps
    res = run_bass_kernel_spmd(nc, in_maps, list(range(N_CORES)))
    _CACHE["last_results"] = res
    out = np.concatenate([res.results[c]["out"] for c in range(N_CORES)],
                         axis=0)
    # [B, 128, NT*C] f16 -> [B, L, C] f32
    return out.reshape(B_FULL, P, NT, C).transpose(0, 2, 1, 3).reshape(
        B_FULL, L, C).astype(np.float32)


def _pjrt_exec(nc, in_maps, n_warm=2, n_time=8):
    """Build a sharded jit for nc, run it, return (best_wall_s, result)."""
    import time
    import jax
    import concourse.mybir as mybir_
    from concourse.bass2jax import (_bass_exec_p, install_neuronx_cc_hook,
                                    partition_id_tensor)
    from jax.experimental.shard_map import shard_map
    from jax.sharding import Mesh, PartitionSpec

    install_neuronx_cc_hook()
    partition_name = (nc.partition_id_tensor.name
                      if nc.partition_id_tensor else None)
    in_names, out_names, out_avals = [], [], []
    for alloc in nc.m.functions[0].allocations:
        if not isinstance(alloc, mybir_.MemoryLocationSet):
            continue
        name = alloc.memorylocations[0].name
        if alloc.kind == "ExternalInput":
            if name != partition_name:
                in_names.append(name)
        elif alloc.kind == "ExternalOutput":
            out_names.append(name)
            out_avals.append(jax.core.ShapedArray(
                tuple(alloc.tensor_shape), mybir_.dt.np(alloc.dtype)))
    n_params = len(in_names)
    all_in_names = list(in_names) + list(out_names)
    if partition_name is not None:
        all_in_names.append(partition_name)

    def _body(*args):
        operands = list(args)
        if partition_name is not None:
            operands.append(partition_id_tensor())
        return tuple(_bass_exec_p.bind(
            *operands,
            out_avals=tuple(out_avals),
            in_names=tuple(all_in_names),
            out_names=tuple(out_names),
            lowering_input_output_aliases=(),
            sim_require_finite=True,
            sim_require_nnan=True,
            nc=nc,
        ))

    devices = jax.devices()[:N_CORES]
    mesh = Mesh(np.asarray(devices), ("core",))
    nin = n_params + len(out_names)
    sharding = jax.sharding.NamedSharding(mesh, PartitionSpec("core"))
    fn = jax.jit(shard_map(
        _body, mesh=mesh,
        in_specs=(PartitionSpec("core"),) * nin,
        out_specs=(PartitionSpec("core"),) * len(out_names),
        check_rep=False))
    dev_args = [
        jax.device_put(np.concatenate(
            [np.asarray(in_maps[c][nm]) for c in range(N_CORES)], axis=0),
            sharding)
        for nm in in_names
    ]
    for av in out_avals:
        z = np.zeros((N_CORES * av.shape[0], *av.shape[1:]), av.dtype)
        dev_args.append(jax.device_put(z, sharding))

    for _ in range(n_warm):
        out = fn(*dev_args)
        jax.block_until_ready(out)
    best = float("inf")
    for _ in range(n_time):
        t0 = time.perf_counter()
        out = fn(*dev_args)
        jax.block_until_ready(out)
        best = min(best, time.perf_counter() - t0)
    result = np.asarray(out[0])
    return best, result


def bench_repeat(reps=8, n_time=10, in_maps=None):
    """Isolate device exec time: time a module doing the work `reps` times
    in-kernel vs once; slope = steady-state HW time per execution."""
    if in_maps is None:
        in_maps = _CACHE["last_in_maps"]
    t1, _ = _pjrt_exec(get_nc(1), in_maps, n_time=n_time)
    tr, result = _pjrt_exec(get_nc(reps), in_maps, n_time=n_time)
    per_exec_ns = (tr - t1) / (reps - 1) * 1e9
    return per_exec_ns, result, t1 * 1e9, tr * 1e9


# revision 15
# speedup vs baseline: 2.0294x; 1.1124x over previous
"""CBAM kernel for Trainium2, 8-way batch-parallel SPMD, f16 data path.

Computes out = x^2 * (att_c[b,c] + sigmoid(conv(spatial_stats))[b,l]) where
att_c = sigmoid(mlp(mean_L x) + mlp(max_L x)), matching the CBAM reference.

Key layout decision: x is staged host-side as float16 in the SBUF tile
layout [NB, 128, NT*C] (partition = l % 128, free col = 256*(l//128) + c),
so each batch loads/stores as ONE dma_start of 128 x 16KB contiguous rows.
This halves HBM traffic vs f32 and collapses ~260 small DMAs into 8 big
ones (the f32 baseline was bottlenecked on per-DMA sequencing overhead,
sync engine 63% busy).

Engine split per batch:
  PE   : channel-sum (x-slices as rhs vs stationary 1/L column), PE
         transposes for channel-major stats, MLP, conv (banded-Toeplitz)
  DVE  : spatial sum+max (3D reduces, f16 4x mode), chan-max fold tree,
         half the final (att + sig) * x^2 scalar_tensor_tensor ops
  ACT  : x^2 squares, sigmoids/relu, psum->sbuf stat copies
  POOL : other half of the final stt ops, output store DMAs
"""

import numpy as np
from contextlib import ExitStack

import concourse.bacc as bacc
import concourse.bass as bass
import concourse.tile as tile
import concourse.mybir as mybir
from concourse.bass_utils import run_bass_kernel_spmd

AF = mybir.ActivationFunctionType
ALU = mybir.AluOpType
AX = mybir.AxisListType
FP32 = mybir.dt.float32
FP16 = mybir.dt.float16

N_CORES = 8
B_FULL = 32
NB = B_FULL // N_CORES  # batches per core = 4
L = 4096
C = 256
HID = 16
P = 128
NT = L // P  # 32 L-tiles per batch
F = NT * C   # 8192 free columns per batch

_CACHE: dict = {}


SATT_DVE = 8      # tiles whose att+sig runs on DVE tensor_scalar (rest ACT)
MUL_DVE = 6656    # columns of the final multiply on DVE (rest Pool)


def _build_body(ctx: ExitStack, tc, out_d, x_d, w1_d, b1_d, w2b_d, cm_d, cc_d,
                ones_d, id_d, rc_d, reps=1):
    nc = tc.nc

    const = ctx.enter_context(tc.tile_pool(name="const", bufs=1))
    xpool = ctx.enter_context(tc.tile_pool(name="x", bufs=2))
    sqpool = ctx.enter_context(tc.tile_pool(name="sq", bufs=3))
    stpool = ctx.enter_context(tc.tile_pool(name="satt", bufs=2))
    opool = ctx.enter_context(tc.tile_pool(name="outt", bufs=2))
    mpool = ctx.enter_context(tc.tile_pool(name="maxtree", bufs=2))
    spool = ctx.enter_context(tc.tile_pool(name="stats", bufs=2))
    apool = ctx.enter_context(tc.tile_pool(name="att", bufs=2))
    pacc = ctx.enter_context(tc.tile_pool(name="pacc", bufs=2, space="PSUM"))
    ptrp = ctx.enter_context(tc.tile_pool(name="ptrp", bufs=2, space="PSUM"))
    pwork = ctx.enter_context(tc.tile_pool(name="pwork", bufs=4, space="PSUM"))

    # param loads ride the scalar queue so the first x load isn't delayed
    w1 = const.tile([P, 2 * (HID + 1)], FP32)
    nc.scalar.dma_start(w1[:], w1_d[:])
    b1 = const.tile([HID + 1, 1], FP32)
    nc.scalar.dma_start(b1[:], b1_d[:])
    w2b = const.tile([HID + 1, C], FP32)
    nc.scalar.dma_start(w2b[:], w2b_d[:])
    cmain = const.tile([P, 2 * P], FP16)
    nc.scalar.dma_start(cmain[:], cm_d[:])
    ccorn = const.tile([P, 4 * P], FP16)
    nc.scalar.dma_start(ccorn[:], cc_d[:])
    ones = const.tile([P, P], FP32)
    nc.scalar.dma_start(ones[:], ones_d[:])
    ident = const.tile([P, P], FP16)
    nc.scalar.dma_start(ident[:], id_d[:])
    redcol = const.tile([P, 1], FP16)
    nc.scalar.dma_start(redcol[:], rc_d[:])

    def spatial_tree(xt, fa, fb, out, op):
        """Per-(p, t) reduce over c via a 2x-mode TT fold tree + small tail.

        Folds c: 256 -> 8 through fa/fb ping-pong, then one cheap 3D reduce.
        """
        x4 = xt[:].rearrange("p (t h c) -> p t h c", h=2, c=128)
        nc.vector.tensor_tensor(
            fa[:, 0:4096].rearrange("p (t h c) -> p t h c", h=1, c=128),
            x4[:, :, 0:1, :], x4[:, :, 1:2, :], op=op)
        seq = [(fa, 4096), (fb, 2048), (fa, 1024), (fb, 512), (fa, 256)]
        for i in range(1, 5):
            sbuf, sw = seq[i - 1]
            dbuf, dw = seq[i]
            ch = sw // NT // 2
            s4 = sbuf[:, 0:sw].rearrange("p (t h c) -> p t h c", h=2, c=ch)
            nc.vector.tensor_tensor(
                dbuf[:, 0:dw].rearrange("p (t h c) -> p t h c", h=1, c=ch),
                s4[:, :, 0:1, :], s4[:, :, 1:2, :], op=op)
        with nc.allow_low_precision(reason="f16 spatial stats feed sigmoid"):
            nc.vector.tensor_reduce(
                out[:], fa[:, 0:256].rearrange("p (t c) -> p t c", c=8),
                axis=AX.X, op=op)

    def emit_stats(b):
        st = {}
        xt = xpool.tile([P, F], FP16, tag="x", name=f"x{b}")
        for h in range(2):
            nc.sync.dma_start(xt[:, F // 2 * h:F // 2 * (h + 1)],
                              x_d[b, :, F // 2 * h:F // 2 * (h + 1)])
        st["xt"] = xt

        # channel sum over L (PE): 16 matmuls of [1, 512]; even tiles land
        # in cols 0:256, odd tiles in 256:512 (folded by DVE below)
        pcs2 = pacc.tile([1, 2 * C], FP32, tag="pcs")
        for j in range(NT // 2):
            nc.tensor.matmul(pcs2[:], redcol[:], xt[:, 2 * C * j:2 * C * (j + 1)],
                             start=(j == 0), stop=(j == NT // 2 - 1),
                             skip_group_check=True)

        # chan-max fold tree (DVE, contiguous halves)
        mb = mpool.tile([P, F // 2], FP16, tag="mb")
        nc.vector.tensor_max(mb[:], xt[:, 0:F // 2], xt[:, F // 2:F])
        w = F // 4
        while w >= C:
            nc.vector.tensor_max(mb[:, 0:w], mb[:, 0:w], mb[:, w:2 * w])
            w //= 2
        # cross-partition max via PE transposes + small DVE reduces
        pmaxT = ptrp.tile([P, 2 * P], FP16, tag="pmaxT")
        for h in range(2):
            nc.tensor.transpose(pmaxT[:, P * h:P * (h + 1)],
                                mb[:, P * h:P * (h + 1)], ident[:])
        stats_cm = spool.tile([P, 4], FP32, tag="stats_cm")
        for h in range(2):
            nc.vector.tensor_reduce(stats_cm[:, 2 * h + 1:2 * h + 2],
                                    pmaxT[:, P * h:P * (h + 1)],
                                    axis=AX.X, op=ALU.max)

        # spatial max + sum trees (DVE)
        fa = mpool.tile([P, 4096], FP16, tag="fa")
        fb = mpool.tile([P, 2048], FP16, tag="fb")
        max_s = spool.tile([P, NT], FP16, tag="max_s")
        spatial_tree(xt, fa, fb, max_s, ALU.max)
        # avg fold: [1, 512] psum -> sbuf copy -> [1, 256] f16
        sr512 = spool.tile([1, 2 * C], FP16, tag="sr512")
        nc.scalar.activation(sr512[:], pcs2[:], AF.Copy)
        avg_row = spool.tile([1, C], FP16, tag="avg_row")
        nc.vector.tensor_add(avg_row[:], sr512[0:1, 0:C], sr512[0:1, C:2 * C])
        sum_s = spool.tile([P, NT], FP16, tag="sum_s")
        spatial_tree(xt, fa, fb, sum_s, ALU.add)

        # avg transposes into channel-major + ACT copies
        pavgT = pwork.tile([P, 4], FP16, tag="pwork")
        for h in range(2):
            nc.tensor.transpose(pavgT[:, 2 * h:2 * h + 1],
                                avg_row[0:1, P * h:P * (h + 1)],
                                ident[0:1, 0:1])
        for h in range(2):
            nc.scalar.activation(stats_cm[:, 2 * h:2 * h + 1],
                                 pavgT[:, 2 * h:2 * h + 1], AF.Copy)

        # shared MLP (f32, tiny): row HID carries the 2*b2 constant
        ph = pwork.tile([HID + 1, 2], FP32, tag="pwork")
        nc.tensor.matmul(ph[:], w1[:, 0:HID + 1], stats_cm[:, 0:2],
                         start=True, stop=False, skip_group_check=True)
        nc.tensor.matmul(ph[:], w1[:, HID + 1:2 * (HID + 1)], stats_cm[:, 2:4],
                         start=False, stop=True, skip_group_check=True)
        hsb = spool.tile([HID + 1, 2], FP32, tag="hsb")
        nc.scalar.activation(hsb[:], ph[:], AF.Relu, bias=b1[:])
        h2 = spool.tile([HID + 1, 1], FP32, tag="h2")
        nc.vector.tensor_add(h2[:], hsb[:, 0:1], hsb[:, 1:2])
        h2r = spool.tile([HID + 1, P], FP32, tag="h2r")
        nc.vector.tensor_scalar_mul(h2r[:], ones[0:HID + 1, :], h2[:])
        po = pwork.tile([P, C], FP32, tag="pwork")
        nc.tensor.matmul(po[:], h2r[:], w2b[:], start=True, stop=True,
                         skip_group_check=True)
        att = apool.tile([P, C], FP16, tag="att")
        nc.scalar.activation(att[:], po[:], AF.Sigmoid)
        st["att"] = att

        # spatial conv over L: banded-Toeplitz matmuls (f16 in)
        pc = pwork.tile([P, NT], FP32, tag="pwork")
        nc.tensor.matmul(pc[:, :], cmain[:, 0:P], sum_s[:],
                         start=True, stop=False, skip_group_check=True)
        nc.tensor.matmul(pc[:, :], cmain[:, P:2 * P], max_s[:],
                         start=False, stop=False, skip_group_check=True)
        nc.tensor.matmul(pc[:, 1:NT], ccorn[:, 0:P], sum_s[:, 0:NT - 1],
                         start=False, stop=False, skip_group_check=True)
        nc.tensor.matmul(pc[:, 1:NT], ccorn[:, P:2 * P], max_s[:, 0:NT - 1],
                         start=False, stop=False, skip_group_check=True)
        nc.tensor.matmul(pc[:, 0:NT - 1], ccorn[0:3, 2 * P:3 * P],
                         sum_s[0:3, 1:NT],
                         start=False, stop=False, skip_group_check=True)
        nc.tensor.matmul(pc[:, 0:NT - 1], ccorn[0:3, 3 * P:4 * P],
                         max_s[0:3, 1:NT],
                         start=False, stop=True, skip_group_check=True)
        sig = apool.tile([P, NT], FP32, tag="sig")
        nc.scalar.activation(sig[:], pc[:], AF.Sigmoid)
        st["sig"] = sig
        st["b"] = b
        return st

    def emit_sq(st):
        # x^2 on ACT, emitted after the previous batch's final so the ACT
        # stats chain (hsb/att/sig) of this batch isn't queued behind it
        sq = sqpool.tile([P, F], FP16, tag="sq")
        xt = st["xt"]
        for h in range(2):
            nc.scalar.activation(sq[:, F // 2 * h:F // 2 * (h + 1)],
                                 xt[:, F // 2 * h:F // 2 * (h + 1)], AF.Square)
        st["sq"] = sq

    def emit_final(st):
        att, sig, sq = st["att"], st["sig"], st["sq"]
        # satt[:, 256t + c] = att[c] + sig[p, t]: DVE tensor_scalar for the
        # first SATT_DVE tiles, ACT identity-with-bias for the rest
        satt = stpool.tile([P, F], FP16, tag="satt")
        for t in range(SATT_DVE):
            nc.vector.tensor_scalar_add(satt[:, C * t:C * (t + 1)], att[:],
                                        sig[:, t:t + 1])
        for t in range(SATT_DVE, NT):
            nc.scalar.activation(satt[:, C * t:C * (t + 1)], att[:],
                                 AF.Identity, bias=sig[:, t:t + 1])
        # out = satt * x^2: one big DVE TT + Pool TT tail
        ot = opool.tile([P, F], FP16, tag="ot")
        nc.vector.tensor_mul(ot[:, 0:MUL_DVE], satt[:, 0:MUL_DVE],
                             sq[:, 0:MUL_DVE])
        nc.gpsimd.tensor_tensor(ot[:, MUL_DVE:F], satt[:, MUL_DVE:F],
                                sq[:, MUL_DVE:F], op=ALU.mult)
        nc.gpsimd.dma_start(out_d[st["b"]], ot[:])

    # software-pipelined emission: final(b-1) lands after stats(b) so no
    # engine stalls head-of-line on the cross-engine satt join; sq(b) goes
    # last so the ACT stats chain isn't queued behind it
    prev = None
    for b in [b for _ in range(reps) for b in range(NB)]:
        cur = emit_stats(b)
        if prev is not None:
            emit_final(prev)
        emit_sq(cur)
        prev = cur
    emit_final(prev)


def _build_nc(reps=1):
    nc = bacc.Bacc("TRN2", target_bir_lowering=False, debug=False,
                   enable_asserts=False, num_devices=N_CORES)
    x_d = nc.dram_tensor("xb", [NB, P, F], FP16, kind="ExternalInput").ap()
    w1_d = nc.dram_tensor("w1sb", [P, 2 * (HID + 1)], FP32, kind="ExternalInput").ap()
    b1_d = nc.dram_tensor("b1col", [HID + 1, 1], FP32, kind="ExternalInput").ap()
    w2b_d = nc.dram_tensor("w2b", [HID + 1, C], FP32, kind="ExternalInput").ap()
    cm_d = nc.dram_tensor("convmain", [P, 2 * P], FP16, kind="ExternalInput").ap()
    cc_d = nc.dram_tensor("convcorner", [P, 4 * P], FP16, kind="ExternalInput").ap()
    ones_d = nc.dram_tensor("ones", [P, P], FP32, kind="ExternalInput").ap()
    id_d = nc.dram_tensor("ident", [P, P], FP16, kind="ExternalInput").ap()
    rc_d = nc.dram_tensor("redcol", [P, 1], FP16, kind="ExternalInput").ap()
    out_d = nc.dram_tensor("out", [NB, P, F], FP16, kind="ExternalOutput").ap()

    with tile.TileContext(nc) as tc:
        with ExitStack() as ctx:
            _build_body(ctx, tc, out_d, x_d, w1_d, b1_d, w2b_d, cm_d, cc_d,
                        ones_d, id_d, rc_d, reps=reps)
    nc.compile()
    return nc


def get_nc(reps=1):
    key = f"nc{reps}"
    if key not in _CACHE:
        _CACHE[key] = _build_nc(reps=reps)
    return _CACHE[key]


def _prep_inputs(W1, b1, W2, b2, conv_w):
    """Host-side parameter preprocessing (shared across cores)."""
    W1 = np.asarray(W1, np.float32)
    W2 = np.asarray(W2, np.float32)
    b1 = np.asarray(b1, np.float32)
    b2 = np.asarray(b2, np.float32)
    conv_w = np.asarray(conv_w, np.float32)

    HB = HID + 1
    w1sb = np.zeros((P, 2 * HB), np.float32)
    for h in range(2):
        w1sb[:, HB * h:HB * h + HID] = W1[P * h:P * (h + 1), :]
    w2b = np.concatenate([W2, b2[None, :]], axis=0).astype(np.float32)
    b1col = np.concatenate([b1, [1.0]]).astype(np.float32).reshape(HB, 1)

    # Banded Toeplitz over two adjacent 128-blocks; avg band folds in the
    # 1/C spatial-mean scale (device computes raw channel sums).
    wa = conv_w[:, 0, 0] / C
    wm = conv_w[:, 1, 0]
    Wb_a = np.zeros((2 * P, 2 * P), np.float32)
    Wb_m = np.zeros((2 * P, 2 * P), np.float32)
    for i in range(2 * P):
        for k in range(7):
            j = i + k - 3
            if 0 <= j < 2 * P:
                Wb_a[i, j] = wa[k]
                Wb_m[i, j] = wm[k]
    cmain = np.concatenate([Wb_a[0:P, 0:P].T, Wb_m[0:P, 0:P].T], axis=1)
    # Corner lhsTs in one [128, 512] tensor. The prev-block ("lo") bands use
    # full K=128 (only rows 125-127 nonzero) so the rhs stays at base
    # partition 0 (PE requires base partition in {0, 32, 64}); the
    # next-block ("hi") bands are K=3 at rows 0-2.
    corn = np.zeros((P, 4 * P), np.float32)
    corn[:, 0:P] = Wb_a[P:2 * P, 0:P].T            # prev-block avg
    corn[:, P:2 * P] = Wb_m[P:2 * P, 0:P].T        # prev-block max
    corn[0:3, 2 * P:3 * P] = Wb_a[0:P, P:2 * P].T[0:3, :]   # next-block avg
    corn[0:3, 3 * P:4 * P] = Wb_m[0:P, P:2 * P].T[0:3, :]   # next-block max
    return {
        "w1sb": w1sb,
        "b1col": np.ascontiguousarray(b1col),
        "w2b": w2b,
        "convmain": np.ascontiguousarray(cmain).astype(np.float16),
        "convcorner": np.ascontiguousarray(corn).astype(np.float16),
        "ones": np.ones((P, P), np.float32),
        "ident": np.eye(P, dtype=np.float16),
        "redcol": np.full((P, 1), 1.0 / L, np.float16),
    }


def kernel(x, W1, b1, W2, b2, conv_w):
    nc = get_nc()
    x = np.asarray(x, np.float32)
    params = _prep_inputs(W1, b1, W2, b2, conv_w)
    # Stage x as f16 in the SBUF tile layout: [NB, 128, NT*C] with
    # col = 256 * (l // 128) + c, partition = l % 128.
    xt = x.reshape(B_FULL, NT, P, C).transpose(0, 2, 1, 3).reshape(
        B_FULL, P, F).astype(np.float16)
    in_maps = []
    for c in range(N_CORES):
        m = dict(params)
        m["xb"] = np.ascontiguousarray(xt[NB * c:NB * (c + 1)])
        in_maps.append(m)
    _CACHE["last_in_maps"] = in_maps
    res = run_bass_kernel_spmd(nc, in_maps, list(range(N_CORES)))
    _CACHE["last_results"] = res
    out = np.concatenate([res.results[c]["out"] for c in range(N_CORES)],
                         axis=0)
    # [B, 128, NT*C] f16 -> [B, L, C] f32
    return out.reshape(B_FULL, P, NT, C).transpose(0, 2, 1, 3).reshape(
        B_FULL, L, C).astype(np.float32)


def _pjrt_exec(nc, in_maps, n_warm=2, n_time=8):
    """Build a sharded jit for nc, run it, return (best_wall_s, result)."""
    import time
    import jax
    import concourse.mybir as mybir_
    from concourse.bass2jax import (_bass_exec_p, install_neuronx_cc_hook,
                                    partition_id_tensor)
    from jax.experimental.shard_map import shard_map
    from jax.sharding import Mesh, PartitionSpec

    install_neuronx_cc_hook()
    partition_name = (nc.partition_id_tensor.name
                      if nc.partition_id_tensor else None)
    in_names, out_names, out_avals = [], [], []
    for alloc in nc.m.functions[0].allocations:
        if not isinstance(alloc, mybir_.MemoryLocationSet):
            continue
        name = alloc.memorylocations[0].name
        if alloc.kind == "ExternalInput":
            if name != partition_name:
                in_names.append(name)
        elif alloc.kind == "ExternalOutput":
            out_names.append(name)
            out_avals.append(jax.core.ShapedArray(
                tuple(alloc.tensor_shape), mybir_.dt.np(alloc.dtype)))
    n_params = len(in_names)
    all_in_names = list(in_names) + list(out_names)
    if partition_name is not None:
        all_in_names.append(partition_name)

    def _body(*args):
        operands = list(args)
        if partition_name is not None:
            operands.append(partition_id_tensor())
        return tuple(_bass_exec_p.bind(
            *operands,
            out_avals=tuple(out_avals),
            in_names=tuple(all_in_names),
            out_names=tuple(out_names),
            lowering_input_output_aliases=(),
            sim_require_finite=True,
            sim_require_nnan=True,
            nc=nc,
        ))

    devices = jax.devices()[:N_CORES]
    mesh = Mesh(np.asarray(devices), ("core",))
    nin = n_params + len(out_names)
    sharding = jax.sharding.NamedSharding(mesh, PartitionSpec("core"))
    fn = jax.jit(shard_map(
        _body, mesh=mesh,
        in_specs=(PartitionSpec("core"),) * nin,
        out_specs=(PartitionSpec("core"),) * len(out_names),
        check_rep=False))
    dev_args = [
        jax.device_put(np.concatenate(
            [np.asarray(in_maps[c][nm]) for c in range(N_CORES)], axis=0),
            sharding)
        for nm in in_names
    ]
    for av in out_avals:
        z = np.zeros((N_CORES * av.shape[0], *av.shape[1:]), av.dtype)
        dev_args.append(jax.device_put(z, sharding))

    for _ in range(n_warm):
        out = fn(*dev_args)
        jax.block_until_ready(out)
    best = float("inf")
    for _ in range(n_time):
        t0 = time.perf_counter()
        out = fn(*dev_args)
        jax.block_until_ready(out)
        best = min(best, time.perf_counter() - t0)
    result = np.asarray(out[0])
    return best, result


def bench_repeat(reps=8, n_time=10, in_maps=None):
    """Isolate device exec time: time a module doing the work `reps` times
    in-kernel vs once; slope = steady-state HW time per execution."""
    if in_maps is None:
        in_maps = _CACHE["last_in_maps"]
    t1, _ = _pjrt_exec(get_nc(1), in_maps, n_time=n_time)
    tr, result = _pjrt_exec(get_nc(reps), in_maps, n_time=n_time)
    per_exec_ns = (tr - t1) / (reps - 1) * 1e9
    return per_exec_ns, result, t1 * 1e9, tr * 1e9


# revision 20
# speedup vs baseline: 2.0545x; 1.0124x over previous
"""CBAM kernel for Trainium2, 8-way batch-parallel SPMD, f16 data path.

Computes out = x^2 * (att_c[b,c] + sigmoid(conv(spatial_stats))[b,l]) where
att_c = sigmoid(mlp(mean_L x) + mlp(max_L x)), matching the CBAM reference.

Key layout decision: x is staged host-side as float16 in the SBUF tile
layout [NB, 128, NT*C] (partition = l % 128, free col = 256*(l//128) + c),
so each batch loads/stores as ONE dma_start of 128 x 16KB contiguous rows.
This halves HBM traffic vs f32 and collapses ~260 small DMAs into 8 big
ones (the f32 baseline was bottlenecked on per-DMA sequencing overhead,
sync engine 63% busy).

Engine split per batch:
  PE   : channel-sum (x-slices as rhs vs stationary 1/L column), PE
         transposes for channel-major stats, MLP, conv (banded-Toeplitz)
  DVE  : spatial sum+max (3D reduces, f16 4x mode), chan-max fold tree,
         half the final (att + sig) * x^2 scalar_tensor_tensor ops
  ACT  : x^2 squares, sigmoids/relu, psum->sbuf stat copies
  POOL : other half of the final stt ops, output store DMAs
"""

import numpy as np
from contextlib import ExitStack

import concourse.bacc as bacc
import concourse.bass as bass
import concourse.tile as tile
import concourse.mybir as mybir
from concourse.bass_utils import run_bass_kernel_spmd

AF = mybir.ActivationFunctionType
ALU = mybir.AluOpType
AX = mybir.AxisListType
FP32 = mybir.dt.float32
FP16 = mybir.dt.float16

N_CORES = 8
B_FULL = 32
NB = B_FULL // N_CORES  # batches per core = 4
L = 4096
C = 256
HID = 16
P = 128
NT = L // P  # 32 L-tiles per batch
F = NT * C   # 8192 free columns per batch

_CACHE: dict = {}


SATT_DVE = 8      # tiles whose att+sig runs on DVE tensor_scalar (rest ACT)
MUL_DVE = 6656    # columns of the final multiply on DVE (rest Pool)


def _build_body(ctx: ExitStack, tc, out_d, x_d, w1_d, b1_d, w2b_d, cm_d, cc_d,
                ones_d, id_d, rc_d, reps=1):
    nc = tc.nc

    const = ctx.enter_context(tc.tile_pool(name="const", bufs=1))
    xpool = ctx.enter_context(tc.tile_pool(name="x", bufs=2))
    sqpool = ctx.enter_context(tc.tile_pool(name="sq", bufs=2))
    stpool = ctx.enter_context(tc.tile_pool(name="satt", bufs=2))
    opool = ctx.enter_context(tc.tile_pool(name="outt", bufs=2))
    mpool = ctx.enter_context(tc.tile_pool(name="maxtree", bufs=2))
    spool = ctx.enter_context(tc.tile_pool(name="stats", bufs=2))
    apool = ctx.enter_context(tc.tile_pool(name="att", bufs=2))
    pacc = ctx.enter_context(tc.tile_pool(name="pacc", bufs=2, space="PSUM"))
    ptrp = ctx.enter_context(tc.tile_pool(name="ptrp", bufs=2, space="PSUM"))
    pwork = ctx.enter_context(tc.tile_pool(name="pwork", bufs=4, space="PSUM"))

    # param loads ride the scalar queue so the first x load isn't delayed
    w1 = const.tile([P, 2 * (HID + 1)], FP32)
    nc.scalar.dma_start(w1[:], w1_d[:])
    b1 = const.tile([HID + 1, 1], FP32)
    nc.scalar.dma_start(b1[:], b1_d[:])
    w2b = const.tile([HID + 1, C], FP32)
    nc.scalar.dma_start(w2b[:], w2b_d[:])
    cmain = const.tile([P, 2 * P], FP16)
    nc.scalar.dma_start(cmain[:], cm_d[:])
    ccorn = const.tile([P, 4 * P], FP16)
    nc.scalar.dma_start(ccorn[:], cc_d[:])
    ones = const.tile([P, P], FP32)
    nc.scalar.dma_start(ones[:], ones_d[:])
    ident = const.tile([P, P], FP16)
    nc.scalar.dma_start(ident[:], id_d[:])
    redcol = const.tile([P, 1], FP16)
    nc.scalar.dma_start(redcol[:], rc_d[:])

    def spatial_fold1(xt, fa, op, half):
        """First c-fold (256 -> 128) for one DMA half of x (16 tiles)."""
        lo = F // 2 * half
        x4 = xt[:, lo:lo + F // 2].rearrange("p (t h c) -> p t h c",
                                             h=2, c=128)
        nc.vector.tensor_tensor(
            fa[:, 2048 * half:2048 * (half + 1)].rearrange(
                "p (t h c) -> p t h c", h=1, c=128),
            x4[:, :, 0:1, :], x4[:, :, 1:2, :], op=op)

    def spatial_rest(fa, fb, out, op):
        """Folds c: 128 -> 8 through fa/fb ping-pong, then a cheap tail."""
        seq = [(fa, 4096), (fb, 2048), (fa, 1024), (fb, 512), (fa, 256)]
        for i in range(1, 5):
            sbuf, sw = seq[i - 1]
            dbuf, dw = seq[i]
            ch = sw // NT // 2
            s4 = sbuf[:, 0:sw].rearrange("p (t h c) -> p t h c", h=2, c=ch)
            nc.vector.tensor_tensor(
                dbuf[:, 0:dw].rearrange("p (t h c) -> p t h c", h=1, c=ch),
                s4[:, :, 0:1, :], s4[:, :, 1:2, :], op=op)
        with nc.allow_low_precision(reason="f16 spatial stats feed sigmoid"):
            nc.vector.tensor_reduce(
                out[:], fa[:, 0:256].rearrange("p (t c) -> p t c", c=8),
                axis=AX.X, op=op)

    def emit_stats(b):
        st = {}
        xt = xpool.tile([P, F], FP16, tag="x", name=f"x{b}")
        for h in range(2):
            nc.sync.dma_start(xt[:, F // 2 * h:F // 2 * (h + 1)],
                              x_d[b, :, F // 2 * h:F // 2 * (h + 1)])
        st["xt"] = xt

        # channel sum over L (PE): 16 matmuls of [1, 512]; even tiles land
        # in cols 0:256, odd tiles in 256:512 (folded by DVE below)
        pcs2 = pacc.tile([1, 2 * C], FP32, tag="pcs")
        for j in range(NT // 2):
            nc.tensor.matmul(pcs2[:], redcol[:], xt[:, 2 * C * j:2 * C * (j + 1)],
                             start=(j == 0), stop=(j == NT // 2 - 1),
                             skip_group_check=True)

        # per-half first folds start as soon as each DMA half lands
        mb = mpool.tile([P, F // 2], FP16, tag="mb")
        fa = mpool.tile([P, 4096], FP16, tag="fa")
        fb = mpool.tile([P, 2048], FP16, tag="fb")
        ga = mpool.tile([P, 4096], FP16, tag="ga")
        gb = mpool.tile([P, 2048], FP16, tag="gb")
        for h in range(2):
            lo = F // 2 * h
            # chan-max: fold tiles {t, t+8} within this half
            nc.vector.tensor_max(mb[:, 2048 * h:2048 * (h + 1)],
                                 xt[:, lo:lo + 2048], xt[:, lo + 2048:lo + 4096])
            spatial_fold1(xt, fa, ALU.max, h)
            spatial_fold1(xt, ga, ALU.add, h)

        # chan-max tree: fold mb 4096 -> 256 (contiguous halves)
        w = F // 4
        while w >= C:
            nc.vector.tensor_max(mb[:, 0:w], mb[:, 0:w], mb[:, w:2 * w])
            w //= 2
        # cross-partition max via PE transposes + small DVE reduces
        pmaxT = ptrp.tile([P, 2 * P], FP16, tag="pmaxT")
        for h in range(2):
            nc.tensor.transpose(pmaxT[:, P * h:P * (h + 1)],
                                mb[:, P * h:P * (h + 1)], ident[:])
        stats_cm = spool.tile([P, 4], FP32, tag="stats_cm")
        for h in range(2):
            nc.vector.tensor_reduce(stats_cm[:, 2 * h + 1:2 * h + 2],
                                    pmaxT[:, P * h:P * (h + 1)],
                                    axis=AX.X, op=ALU.max)

        # spatial max + sum trees (DVE)
        max_s = spool.tile([P, NT], FP16, tag="max_s")
        spatial_rest(fa, fb, max_s, ALU.max)
        # avg fold: [1, 512] psum -> sbuf copy -> [1, 256] f16
        sr512 = spool.tile([1, 2 * C], FP16, tag="sr512")
        nc.scalar.activation(sr512[:], pcs2[:], AF.Copy)
        avg_row = spool.tile([1, C], FP16, tag="avg_row")
        nc.vector.tensor_add(avg_row[:], sr512[0:1, 0:C], sr512[0:1, C:2 * C])
        sum_s = spool.tile([P, NT], FP16, tag="sum_s")
        spatial_rest(ga, gb, sum_s, ALU.add)

        # avg transposes into channel-major + ACT copies
        pavgT = pwork.tile([P, 4], FP16, tag="pwork")
        for h in range(2):
            nc.tensor.transpose(pavgT[:, 2 * h:2 * h + 1],
                                avg_row[0:1, P * h:P * (h + 1)],
                                ident[0:1, 0:1])
        for h in range(2):
            nc.scalar.activation(stats_cm[:, 2 * h:2 * h + 1],
                                 pavgT[:, 2 * h:2 * h + 1], AF.Copy)

        # shared MLP (f32, tiny): row HID carries the 2*b2 constant
        ph = pwork.tile([HID + 1, 2], FP32, tag="pwork")
        nc.tensor.matmul(ph[:], w1[:, 0:HID + 1], stats_cm[:, 0:2],
                         start=True, stop=False, skip_group_check=True)
        nc.tensor.matmul(ph[:], w1[:, HID + 1:2 * (HID + 1)], stats_cm[:, 2:4],
                         start=False, stop=True, skip_group_check=True)
        hsb = spool.tile([HID + 1, 2], FP32, tag="hsb")
        nc.scalar.activation(hsb[:], ph[:], AF.Relu, bias=b1[:])
        h2 = spool.tile([HID + 1, 1], FP32, tag="h2")
        nc.vector.tensor_add(h2[:], hsb[:, 0:1], hsb[:, 1:2])
        h2r = spool.tile([HID + 1, P], FP32, tag="h2r")
        nc.vector.tensor_scalar_mul(h2r[:], ones[0:HID + 1, :], h2[:])
        po = pwork.tile([P, C], FP32, tag="pwork")
        nc.tensor.matmul(po[:], h2r[:], w2b[:], start=True, stop=True,
                         skip_group_check=True)
        att = apool.tile([P, C], FP16, tag="att")
        nc.scalar.activation(att[:], po[:], AF.Sigmoid)
        st["att"] = att

        # spatial conv over L: banded-Toeplitz matmuls (f16 in)
        pc = pwork.tile([P, NT], FP32, tag="pwork")
        nc.tensor.matmul(pc[:, :], cmain[:, 0:P], sum_s[:],
                         start=True, stop=False, skip_group_check=True)
        nc.tensor.matmul(pc[:, :], cmain[:, P:2 * P], max_s[:],
                         start=False, stop=False, skip_group_check=True)
        nc.tensor.matmul(pc[:, 1:NT], ccorn[:, 0:P], sum_s[:, 0:NT - 1],
                         start=False, stop=False, skip_group_check=True)
        nc.tensor.matmul(pc[:, 1:NT], ccorn[:, P:2 * P], max_s[:, 0:NT - 1],
                         start=False, stop=False, skip_group_check=True)
        nc.tensor.matmul(pc[:, 0:NT - 1], ccorn[0:3, 2 * P:3 * P],
                         sum_s[0:3, 1:NT],
                         start=False, stop=False, skip_group_check=True)
        nc.tensor.matmul(pc[:, 0:NT - 1], ccorn[0:3, 3 * P:4 * P],
                         max_s[0:3, 1:NT],
                         start=False, stop=True, skip_group_check=True)
        sig = apool.tile([P, NT], FP32, tag="sig")
        nc.scalar.activation(sig[:], pc[:], AF.Sigmoid)
        st["sig"] = sig
        st["b"] = b
        return st

    def emit_sq(st):
        # x^2 on ACT, emitted after the previous batch's final so the ACT
        # stats chain (hsb/att/sig) of this batch isn't queued behind it
        sq = sqpool.tile([P, F], FP16, tag="sq")
        xt = st["xt"]
        for h in range(2):
            nc.scalar.activation(sq[:, F // 2 * h:F // 2 * (h + 1)],
                                 xt[:, F // 2 * h:F // 2 * (h + 1)], AF.Square)
        st["sq"] = sq

    def emit_final(st, last=False):
        att, sig, sq = st["att"], st["sig"], st["sq"]
        # satt[:, 256t + c] = att[c] + sig[p, t]: DVE tensor_scalar for the
        # first satt_dve tiles, ACT identity-with-bias for the rest
        satt_dve = NT // 2 if last else SATT_DVE
        mul_dve = F - 1024 if last else MUL_DVE
        satt = stpool.tile([P, F], FP16, tag="satt")
        for t in range(satt_dve):
            nc.vector.tensor_scalar_add(satt[:, C * t:C * (t + 1)], att[:],
                                        sig[:, t:t + 1])
        for t in range(satt_dve, NT):
            nc.scalar.activation(satt[:, C * t:C * (t + 1)], att[:],
                                 AF.Identity, bias=sig[:, t:t + 1])
        # out = satt * x^2: one big DVE TT + Pool TT tail
        ot = opool.tile([P, F], FP16, tag="ot")
        nc.vector.tensor_mul(ot[:, 0:mul_dve], satt[:, 0:mul_dve],
                             sq[:, 0:mul_dve])
        nc.gpsimd.tensor_tensor(ot[:, mul_dve:F], satt[:, mul_dve:F],
                                sq[:, mul_dve:F], op=ALU.mult)
        nc.gpsimd.dma_start(out_d[st["b"]], ot[:])

    # software-pipelined emission: final(b-1) lands after stats(b) so no
    # engine stalls head-of-line on the cross-engine satt join; sq(b) goes
    # last so the ACT stats chain isn't queued behind it
    prev = None
    for b in [b for _ in range(reps) for b in range(NB)]:
        cur = emit_stats(b)
        if prev is not None:
            emit_final(prev)
        emit_sq(cur)
        prev = cur
    emit_final(prev, last=True)


def _build_nc(reps=1):
    nc = bacc.Bacc("TRN2", target_bir_lowering=False, debug=False,
                   enable_asserts=False, num_devices=N_CORES)
    x_d = nc.dram_tensor("xb", [NB, P, F], FP16, kind="ExternalInput").ap()
    w1_d = nc.dram_tensor("w1sb", [P, 2 * (HID + 1)], FP32, kind="ExternalInput").ap()
    b1_d = nc.dram_tensor("b1col", [HID + 1, 1], FP32, kind="ExternalInput").ap()
    w2b_d = nc.dram_tensor("w2b", [HID + 1, C], FP32, kind="ExternalInput").ap()
    cm_d = nc.dram_tensor("convmain", [P, 2 * P], FP16, kind="ExternalInput").ap()
    cc_d = nc.dram_tensor("convcorner", [P, 4 * P], FP16, kind="ExternalInput").ap()
    ones_d = nc.dram_tensor("ones", [P, P], FP32, kind="ExternalInput").ap()
    id_d = nc.dram_tensor("ident", [P, P], FP16, kind="ExternalInput").ap()
    rc_d = nc.dram_tensor("redcol", [P, 1], FP16, kind="ExternalInput").ap()
    out_d = nc.dram_tensor("out", [NB, P, F], FP16, kind="ExternalOutput").ap()

    with tile.TileContext(nc) as tc:
        with ExitStack() as ctx:
            _build_body(ctx, tc, out_d, x_d, w1_d, b1_d, w2b_d, cm_d, cc_d,
                        ones_d, id_d, rc_d, reps=reps)
    nc.compile()
    return nc


def get_nc(reps=1):
    key = f"nc{reps}"
    if key not in _CACHE:
        _CACHE[key] = _build_nc(reps=reps)
    return _CACHE[key]


def _prep_inputs(W1, b1, W2, b2, conv_w):
    """Host-side parameter preprocessing (shared across cores)."""
    W1 = np.asarray(W1, np.float32)
    W2 = np.asarray(W2, np.float32)
    b1 = np.asarray(b1, np.float32)
    b2 = np.asarray(b2, np.float32)
    conv_w = np.asarray(conv_w, np.float32)

    HB = HID + 1
    w1sb = np.zeros((P, 2 * HB), np.float32)
    for h in range(2):
        w1sb[:, HB * h:HB * h + HID] = W1[P * h:P * (h + 1), :]
    w2b = np.concatenate([W2, b2[None, :]], axis=0).astype(np.float32)
    b1col = np.concatenate([b1, [1.0]]).astype(np.float32).reshape(HB, 1)

    # Banded Toeplitz over two adjacent 128-blocks; avg band folds in the
    # 1/C spatial-mean scale (device computes raw channel sums).
    wa = conv_w[:, 0, 0] / C
    wm = conv_w[:, 1, 0]
    Wb_a = np.zeros((2 * P, 2 * P), np.float32)
    Wb_m = np.zeros((2 * P, 2 * P), np.float32)
    for i in range(2 * P):
        for k in range(7):
            j = i + k - 3
            if 0 <= j < 2 * P:
                Wb_a[i, j] = wa[k]
                Wb_m[i, j] = wm[k]
    cmain = np.concatenate([Wb_a[0:P, 0:P].T, Wb_m[0:P, 0:P].T], axis=1)
    # Corner lhsTs in one [128, 512] tensor. The prev-block ("lo") bands use
    # full K=128 (only rows 125-127 nonzero) so the rhs stays at base
    # partition 0 (PE requires base partition in {0, 32, 64}); the
    # next-block ("hi") bands are K=3 at rows 0-2.
    corn = np.zeros((P, 4 * P), np.float32)
    corn[:, 0:P] = Wb_a[P:2 * P, 0:P].T            # prev-block avg
    corn[:, P:2 * P] = Wb_m[P:2 * P, 0:P].T        # prev-block max
    corn[0:3, 2 * P:3 * P] = Wb_a[0:P, P:2 * P].T[0:3, :]   # next-block avg
    corn[0:3, 3 * P:4 * P] = Wb_m[0:P, P:2 * P].T[0:3, :]   # next-block max
    return {
        "w1sb": w1sb,
        "b1col": np.ascontiguousarray(b1col),
        "w2b": w2b,
        "convmain": np.ascontiguousarray(cmain).astype(np.float16),
        "convcorner": np.ascontiguousarray(corn).astype(np.float16),
        "ones": np.ones((P, P), np.float32),
        "ident": np.eye(P, dtype=np.float16),
        "redcol": np.full((P, 1), 1.0 / L, np.float16),
    }


def kernel(x, W1, b1, W2, b2, conv_w):
    nc = get_nc()
    x = np.asarray(x, np.float32)
    params = _prep_inputs(W1, b1, W2, b2, conv_w)
    # Stage x as f16 in the SBUF tile layout: [NB, 128, NT*C] with
    # col = 256 * (l // 128) + c, partition = l % 128.
    xt = x.reshape(B_FULL, NT, P, C).transpose(0, 2, 1, 3).reshape(
        B_FULL, P, F).astype(np.float16)
    in_maps = []
    for c in range(N_CORES):
        m = dict(params)
        m["xb"] = np.ascontiguousarray(xt[NB * c:NB * (c + 1)])
        in_maps.append(m)
    _CACHE["last_in_maps"] = in_maps
    res = run_bass_kernel_spmd(nc, in_maps, list(range(N_CORES)))
    _CACHE["last_results"] = res
    out = np.concatenate([res.results[c]["out"] for c in range(N_CORES)],
                         axis=0)
    # [B, 128, NT*C] f16 -> [B, L, C] f32
    return out.reshape(B_FULL, P, NT, C).transpose(0, 2, 1, 3).reshape(
        B_FULL, L, C).astype(np.float32)


def _pjrt_exec(nc, in_maps, n_warm=2, n_time=8):
    """Build a sharded jit for nc, run it, return (best_wall_s, result)."""
    import time
    import jax
    import concourse.mybir as mybir_
    from concourse.bass2jax import (_bass_exec_p, install_neuronx_cc_hook,
                                    partition_id_tensor)
    from jax.experimental.shard_map import shard_map
    from jax.sharding import Mesh, PartitionSpec

    install_neuronx_cc_hook()
    partition_name = (nc.partition_id_tensor.name
                      if nc.partition_id_tensor else None)
    in_names, out_names, out_avals = [], [], []
    for alloc in nc.m.functions[0].allocations:
        if not isinstance(alloc, mybir_.MemoryLocationSet):
            continue
        name = alloc.memorylocations[0].name
        if alloc.kind == "ExternalInput":
            if name != partition_name:
                in_names.append(name)
        elif alloc.kind == "ExternalOutput":
            out_names.append(name)
            out_avals.append(jax.core.ShapedArray(
                tuple(alloc.tensor_shape), mybir_.dt.np(alloc.dtype)))
    n_params = len(in_names)
    all_in_names = list(in_names) + list(out_names)
    if partition_name is not None:
        all_in_names.append(partition_name)

    def _body(*args):
        operands = list(args)
        if partition_name is not None:
            operands.append(partition_id_tensor())
        return tuple(_bass_exec_p.bind(
            *operands,
            out_avals=tuple(out_avals),
            in_names=tuple(all_in_names),
            out_names=tuple(out_names),
            lowering_input_output_aliases=(),
            sim_require_finite=True,
            sim_require_nnan=True,
            nc=nc,
        ))

    devices = jax.devices()[:N_CORES]
    mesh = Mesh(np.asarray(devices), ("core",))
    nin = n_params + len(out_names)
    sharding = jax.sharding.NamedSharding(mesh, PartitionSpec("core"))
    fn = jax.jit(shard_map(
        _body, mesh=mesh,
        in_specs=(PartitionSpec("core"),) * nin,
        out_specs=(PartitionSpec("core"),) * len(out_names),
        check_rep=False))
    dev_args = [
        jax.device_put(np.concatenate(
            [np.asarray(in_maps[c][nm]) for c in range(N_CORES)], axis=0),
            sharding)
        for nm in in_names
    ]
    for av in out_avals:
        z = np.zeros((N_CORES * av.shape[0], *av.shape[1:]), av.dtype)
        dev_args.append(jax.device_put(z, sharding))

    for _ in range(n_warm):
        out = fn(*dev_args)
        jax.block_until_ready(out)
    best = float("inf")
    for _ in range(n_time):
        t0 = time.perf_counter()
        out = fn(*dev_args)
        jax.block_until_ready(out)
        best = min(best, time.perf_counter() - t0)
    result = np.asarray(out[0])
    return best, result


def bench_repeat(reps=8, n_time=10, in_maps=None):
    """Isolate device exec time: time a module doing the work `reps` times
    in-kernel vs once; slope = steady-state HW time per execution."""
    if in_maps is None:
        in_maps = _CACHE["last_in_maps"]
    t1, _ = _pjrt_exec(get_nc(1), in_maps, n_time=n_time)
    tr, result = _pjrt_exec(get_nc(reps), in_maps, n_time=n_time)
    per_exec_ns = (tr - t1) / (reps - 1) * 1e9
    return per_exec_ns, result, t1 * 1e9, tr * 1e9
